# revision 1
# baseline (speedup 1.0000x reference)
"""Trainium2 Bass kernel for nn_AttnBlock (dense transformer block).

Strategy (pure data-parallel over batch, 8 cores):
  - Each core gets B/8 = 512 samples; all weights replicated.
  - Algebraic fusion (host-side, weights only):
      * attention applied to raw x:  y_h = attn_h @ x    (per sample)
      * V-projection and output projection fused: Wvp_h = Wv_h @ Wp_h
      * per-head bias term folded in as extra contraction rows using the
        gate vector:  proj += gate[b,:] @ (bv_h @ Wp_h)
      * softmax normalization (1/Z) and gate applied as one row-scale
        between the attention matmul and the fused projection.
  - All dense GEMMs run in "T-layout" (features on partitions, rows on
    free dim) with weights as the stationary operand; bf16 operands with
    fp32 PSUM accumulation.
  - Per-sample attention matmuls are batched 12 samples at a time using
    block-diagonal [120,120] attention tiles so the PE runs dense work.

Self-contained: hardcodes shapes; imports only the concourse stack.
"""

import math
import os
import sys

import numpy as np

for _p in ("/opt/trn_rl_repo", os.path.expanduser("~/.axon_site/_ro/trn_rl_repo")):
    if os.path.isdir(_p) and _p not in sys.path:
        sys.path.insert(0, _p)

import ml_dtypes  # noqa: E402

import concourse.bass as bass  # noqa: E402
import concourse.mybir as mybir  # noqa: E402
import concourse.tile as tile  # noqa: E402
from concourse import bacc  # noqa: E402
from concourse.masks import make_identity  # noqa: E402

F32 = mybir.dt.float32
BF16 = mybir.dt.bfloat16
F32R = mybir.dt.float32r
AF = mybir.ActivationFunctionType
ALU = mybir.AluOpType

# Problem shapes (hardcoded per spec)
B, S, F, D, H = 4096, 10, 512, 512, 4
EPS = 1e-5
NCORES = 8
BC = B // NCORES          # samples per core = 512
P = 128

# Tiling
C = 32                    # samples per chunk
NCH = BC // C             # 16 chunks
MC = C * S                # 320 rows per chunk
WINS = (12, 12, 8)        # samples per attention window (sum = C)
FT = F // P               # 4 input-feature tiles
TT = (H * D) // P         # 16 q/k output tiles
D1T = (4 * D) // P        # 16 ffn hidden tiles
DPT = D // P              # 4 d_model tiles


def build_kernel(apply_ln_affine: bool, nch: int = NCH, debug: bool = False):
    MR = nch * MC  # rows handled by this program
    nc = bacc.Bacc(None, target_bir_lowering=False, debug=debug)
    names = {}

    _lp = nc.allow_low_precision(reason="float32r intermediates are 4-byte")
    _lp.__enter__()
    with tile.TileContext(nc) as tc:
        with tc.tile_pool(name="dram", bufs=1, space="DRAM") as dram:
            # per-core inputs (bf16 x, prepared on host)
            x_bf = dram.tile([MR, F], BF16, kind="ExternalInput", name="x_bf", uniquify=False)
            # prepared weights (host-fused / pre-scaled), all bf16
            wq_d = dram.tile([F, H * D], BF16, kind="ExternalInput", name="wq_p", uniquify=False)
            wk_d = dram.tile([F, H * D], BF16, kind="ExternalInput", name="wk_p", uniquify=False)
            wvp_d = dram.tile([H * F, D], BF16, kind="ExternalInput", name="wvp_p", uniquify=False)
            w1_d = dram.tile([D, 4 * D], BF16, kind="ExternalInput", name="w1_p", uniquify=False)
            w2_d = dram.tile([4 * D, D], BF16, kind="ExternalInput", name="w2_p", uniquify=False)
            wg_d = dram.tile([F, H], BF16, kind="ExternalInput", name="wg_p", uniquify=False)
            cg_d = dram.tile([H, D], BF16, kind="ExternalInput", name="cg_p", uniquify=False)
            bqc_d = dram.tile([P, TT], F32, kind="ExternalInput", name="bqc_p", uniquify=False)
            bkc_d = dram.tile([P, TT], F32, kind="ExternalInput", name="bkc_p", uniquify=False)
            bpc_d = dram.tile([P, DPT], F32, kind="ExternalInput", name="bpc_p", uniquify=False)
            b1c_d = dram.tile([P, D1T], F32, kind="ExternalInput", name="b1c_p", uniquify=False)
            b2c_d = dram.tile([P, DPT], F32, kind="ExternalInput", name="b2c_p", uniquify=False)
            bg_d = dram.tile([1, H], BF16, kind="ExternalInput", name="bg_p", uniquify=False)
            mask_d = dram.tile([120, 120], BF16, kind="ExternalInput", name="mask_p", uniquify=False)
            if apply_ln_affine:
                ln_d = dram.tile([4, D], F32, kind="ExternalInput", name="ln_p", uniquify=False)
            out_d = dram.tile([MR, F], F32, kind="ExternalOutput", name="out", uniquify=False)
        names["out"] = "out"

        from contextlib import ExitStack
        _stack = ExitStack()
        const = _stack.enter_context(tc.tile_pool(name="const", bufs=1))
        wts = _stack.enter_context(tc.tile_pool(name="wts", bufs=1))
        act = _stack.enter_context(tc.tile_pool(name="act", bufs=1))
        f32w = _stack.enter_context(tc.tile_pool(name="f32w", bufs=1))
        psq = _stack.enter_context(tc.tile_pool(name="psq", bufs=2, space="PSUM"))
        psb = _stack.enter_context(tc.tile_pool(name="psb", bufs=4, space="PSUM"))
        psr = _stack.enter_context(tc.tile_pool(name="psr", bufs=2, space="PSUM"))

        # ---- constants ----
        ident = const.tile([P, P], F32, tag="ident")
        make_identity(nc, ident)
        ones_row_bf = const.tile([1, 512], BF16, tag="ones_row_bf")
        nc.vector.memset(ones_row_bf[:], 1.0)
        ones_tmp = const.tile([P, P], F32, tag="ones_tmp")
        nc.vector.memset(ones_tmp[:], 1.0)
        ones_row_f32 = const.tile([1, P], F32R, tag="ones_row_f32")
        nc.vector.tensor_copy(ones_row_f32[:], ones_tmp[0:1, :])
        ones_col_f32 = const.tile([P, 1], F32R, tag="ones_col_f32")
        nc.vector.tensor_copy(ones_col_f32[:], ones_tmp[:, 0:1])
        ones_col_bf = const.tile([P, 1], BF16, tag="ones_col_bf")
        nc.vector.memset(ones_col_bf[:], 1.0)
        eps_sb = const.tile([1, 1], F32, tag="eps")
        nc.vector.memset(eps_sb[:], EPS)
        mask_bd = const.tile([120, 120], BF16, tag="mask_bd")
        nc.gpsimd.dma_start(mask_bd[:], mask_d[:])

        # ---- resident weights ----
        wq_sb = wts.tile([P, FT, H * D], BF16, tag="wq")
        wk_sb = wts.tile([P, FT, H * D], BF16, tag="wk")
        wvp_sb = wts.tile([P, TT, D], BF16, tag="wvp")
        w1_sb = wts.tile([P, FT, 4 * D], BF16, tag="w1")
        w2_sb = wts.tile([P, D1T, D], BF16, tag="w2")
        wg_sb = wts.tile([P, FT, H], BF16, tag="wg")
        cg_sb = wts.tile([H, D], BF16, tag="cg")
        bqc_sb = wts.tile([P, TT], F32, tag="bqc")
        bkc_sb = wts.tile([P, TT], F32, tag="bkc")
        bpc_sb = wts.tile([P, DPT], F32, tag="bpc")
        b1c_sb = wts.tile([P, D1T], F32, tag="b1c")
        b2c_sb = wts.tile([P, DPT], F32, tag="b2c")
        bg_sb = wts.tile([1, H], BF16, tag="bg")
        # small tensors first so early chunks aren't blocked behind big DMAs
        nc.sync.dma_start(bqc_sb[:], bqc_d[:])
        nc.sync.dma_start(bkc_sb[:], bkc_d[:])
        nc.sync.dma_start(bpc_sb[:], bpc_d[:])
        nc.sync.dma_start(b1c_sb[:], b1c_d[:])
        nc.sync.dma_start(b2c_sb[:], b2c_d[:])
        nc.sync.dma_start(bg_sb[:], bg_d[:])
        nc.sync.dma_start(wg_sb[:], wg_d[:].rearrange("(t p) n -> p t n", p=P))
        nc.sync.dma_start(cg_sb[:], cg_d[:])
        nc.sync.dma_start(wq_sb[:], wq_d[:].rearrange("(t p) n -> p t n", p=P))
        nc.sync.dma_start(wk_sb[:], wk_d[:].rearrange("(t p) n -> p t n", p=P))
        nc.sync.dma_start(wvp_sb[:], wvp_d[:].rearrange("(t p) n -> p t n", p=P))
        nc.sync.dma_start(w1_sb[:], w1_d[:].rearrange("(t p) n -> p t n", p=P))
        nc.sync.dma_start(w2_sb[:], w2_d[:].rearrange("(t p) n -> p t n", p=P))
        if apply_ln_affine:
            ln_sb = wts.tile([P, 4, DPT], F32, tag="ln")
            # ln_d rows: g1, be1, g2, be2 ; [512] -> [p, dpt]
            nc.sync.dma_start(
                ln_sb[:], ln_d[:].rearrange("r (t p) -> p r t", p=P)
            )

        def evac_engine(i):
            return nc.vector if (i % 2 == 0) else nc.scalar

        def copy_out(eng, dst, src):
            if eng is nc.vector:
                nc.vector.tensor_copy(dst, src)
            else:
                nc.scalar.copy(dst, src)

        x_flat = x_bf[:]  # [5120, 512] bf16 dram
        out_flat = out_d[:]

        for ch in range(nch):
            m0 = ch * MC

            # ---- load x: T-layout via DMA transpose; windows in row layout
            xt = act.tile([P, FT, MC], BF16, tag="xt", bufs=2)
            for ft in range(FT):
                nc.scalar.dma_start(
                    xt[:, ft, :],
                    x_flat[m0:m0 + MC, ft * P:(ft + 1) * P],
                    transpose=True,
                )
            xw = act.tile([P, len(WINS), F], BF16, tag="xw", bufs=2)
            wo = 0
            for w, wn in enumerate(WINS):
                nc.gpsimd.dma_start(
                    xw[:wn * S, w, :], x_flat[m0 + wo * S:m0 + (wo + wn) * S, :]
                )
                wo += wn

            # ---- Q/K projections (T-layout out, bf16) ----
            qt = act.tile([P, TT, MC], BF16, tag="qt")
            kt = act.tile([P, TT, MC], BF16, tag="kt")
            for which, (wsb, bsb, dst) in enumerate(
                ((wq_sb, bqc_sb, qt), (wk_sb, bkc_sb, kt))
            ):
                for t in range(TT):
                    ps = psq.tile([P, 512], F32, tag="qk")
                    for ft in range(FT):
                        nc.tensor.matmul(
                            ps[:, :MC],
                            lhsT=wsb[:, ft, t * P:(t + 1) * P],
                            rhs=xt[:, ft, :],
                            start=(ft == 0),
                            stop=(ft == FT - 1),
                        )
                    nc.vector.tensor_scalar_add(
                        dst[:, t, :], ps[:, :MC], bsb[:, t:t + 1]
                    )

            # ---- gate: softmax(x.mean(1) @ Wg + bg) ----
            xm_bf = act.tile([P, FT, C], BF16, tag="xm", bufs=2)
            for ft in range(FT):
                xm = f32w.tile([P, C], F32, tag="xmf", bufs=2)
                nc.vector.tensor_reduce(
                    xm[:, :],
                    xt[:, ft, :].rearrange("p (b s) -> p b s", s=S),
                    axis=mybir.AxisListType.X,
                    op=ALU.add,
                )
                nc.vector.tensor_copy(xm_bf[:, ft, :], xm[:, :])
            psg = psr.tile([C, H], F32, tag="rows")
            for ft in range(FT):
                nc.tensor.matmul(
                    psg[:, :],
                    lhsT=xm_bf[:, ft, :],
                    rhs=wg_sb[:, ft, :],
                    start=(ft == 0),
                    stop=False,
                )
            nc.tensor.matmul(
                psg[:, :],
                lhsT=ones_row_bf[0:1, :C],
                rhs=bg_sb[0:1, :],
                start=False,
                stop=True,
            )
            eg = f32w.tile([C, H], F32, tag="eg", bufs=2)
            zg = f32w.tile([C, 1], F32, tag="zg", bufs=2)
            nc.scalar.activation(eg[:, :], psg[:, :], AF.Exp, accum_out=zg[:, :])
            rzg = f32w.tile([C, 1], F32, tag="rzg", bufs=2)
            nc.vector.reciprocal(rzg[:, :], zg[:, :])
            gatef = f32w.tile([C, H], F32, tag="gatef", bufs=2)
            nc.vector.tensor_scalar_mul(gatef[:, :], eg[:, :], rzg[:, :])
            # transpose gate -> [H, C] then replicate over s -> [H, MC] bf16
            psgt = psr.tile([H, C], F32, tag="rows")
            nc.tensor.transpose(psgt[:, :], gatef[:, :], ident[:C, :C])
            gft = f32w.tile([H, C], F32, tag="gft", bufs=2)
            nc.vector.tensor_copy(gft[:, :], psgt[:, :])
            grep = act.tile([H, MC], BF16, tag="grep", bufs=2)
            for s in range(S):
                nc.vector.tensor_copy(
                    grep[:, :].rearrange("h (b s) -> h b s", s=S)[:, :, s], gft[:, :]
                )

            # ---- attention windows ----
            ysc = act.tile([P, TT, MC], BF16, tag="ysc")
            wo = 0
            for w, wn in enumerate(WINS):
                L = wn * S
                psz = psr.tile([1, 512], F32, tag="rows")
                atts = []
                for h in range(H):
                    pss = psq.tile([P, 512], F32, tag="qk")
                    for dt in range(FT):
                        t = h * FT + dt
                        nc.tensor.matmul(
                            pss[:L, :L],
                            lhsT=kt[:, t, wo * S:wo * S + L],
                            rhs=qt[:, t, wo * S:wo * S + L],
                            start=(dt == 0),
                            stop=(dt == FT - 1),
                        )
                    es = act.tile([120, 128], BF16, tag="es", bufs=6)
                    nc.scalar.activation(es[:L, :L], pss[:L, :L], AF.Exp)
                    abd = act.tile([120, 128], BF16, tag="abd", bufs=8)
                    nc.vector.tensor_mul(abd[:L, :L], es[:L, :L], mask_bd[:L, :L])
                    nc.tensor.matmul(
                        psz[0:1, h * L:h * L + L],
                        lhsT=ones_col_bf[:L, 0:1],
                        rhs=abd[:L, :L],
                        start=True,
                        stop=True,
                    )
                    atts.append(abd)
                # w_row = gate/(Z) in (h, bw, s) order
                grow = act.tile([1, 512], BF16, tag="grow", bufs=3)
                nc.gpsimd.dma_start(grow[0:1, :H * L], grep[:, wo * S:wo * S + L])
                rz = f32w.tile([1, 512], F32, tag="rz", bufs=1)
                nc.vector.reciprocal(rz[0:1, :H * L], psz[0:1, :H * L])
                wrow = f32w.tile([1, 512], F32R, tag="wrow", bufs=1)
                nc.vector.tensor_mul(
                    wrow[0:1, :H * L], rz[0:1, :H * L], grow[0:1, :H * L]
                )
                psw = psb.tile([P, 512], F32, tag="big")
                nc.tensor.matmul(
                    psw[:, :H * L],
                    lhsT=ones_row_f32[0:1, :],
                    rhs=wrow[0:1, :H * L],
                    start=True,
                    stop=True,
                )
                wbc = f32w.tile([P, 512], F32, tag="wbc", bufs=2)
                nc.scalar.copy(wbc[:, :H * L], psw[:, :H * L])
                for ft in range(FT):
                    psy = psb.tile([P, 512], F32, tag="big")
                    for h in range(H):
                        nc.tensor.matmul(
                            psy[:, h * L:h * L + L],
                            lhsT=xw[:L, w, ft * P:(ft + 1) * P],
                            rhs=atts[h][:L, :L],
                            start=True,
                            stop=True,
                        )
                    # scale by w_row and scatter into ysc[(h*FT+ft)]
                    nc.vector.tensor_mul(
                        ysc[:, :, wo * S:wo * S + L]
                        .rearrange("p (h f) m -> p h f m", f=FT)[:, :, ft, :],
                        psy[:, :H * L].rearrange("p (h m) -> p h m", m=L),
                        wbc[:, :H * L].rearrange("p (h m) -> p h m", m=L),
                    )
                wo += wn

            # ---- fused projection + residual ----
            x1u = f32w.tile([P, DPT, MC], F32R, tag="x1u")
            for dp in range(DPT):
                ps = psb.tile([P, 512], F32, tag="big")
                for t in range(TT):
                    nc.tensor.matmul(
                        ps[:, :MC],
                        lhsT=wvp_sb[:, t, dp * P:(dp + 1) * P],
                        rhs=ysc[:, t, :],
                        start=(t == 0),
                        stop=False,
                    )
                nc.tensor.matmul(
                    ps[:, :MC],
                    lhsT=cg_sb[:, dp * P:(dp + 1) * P],
                    rhs=grep[:, :],
                    start=False,
                    stop=True,
                )
                nc.vector.scalar_tensor_tensor(
                    out=x1u[:, dp, :],
                    in0=ps[:, :MC],
                    scalar=bpc_sb[:, dp:dp + 1],
                    in1=xt[:, dp, :],
                    op0=ALU.add,
                    op1=ALU.add,
                )

            # ---- layernorm helper (T-layout, stats via PE ones-matmul) ----
            def layernorm(xu, xn_dst, ln_row):
                sq = f32w.tile([P, DPT, MC], F32R, tag="sq")
                for dp in range(DPT):
                    nc.scalar.activation(sq[:, dp, :], xu[:, dp, :], AF.Square)
                pssum = psr.tile([1, 512], F32, tag="rows")
                pssq = psr.tile([1, 512], F32, tag="rows")
                for dp in range(DPT):
                    nc.tensor.matmul(
                        pssum[0:1, :MC],
                        lhsT=ones_col_f32[:, 0:1],
                        rhs=xu[:, dp, :],
                        start=(dp == 0),
                        stop=(dp == DPT - 1),
                    )
                    nc.tensor.matmul(
                        pssq[0:1, :MC],
                        lhsT=ones_col_f32[:, 0:1],
                        rhs=sq[:, dp, :],
                        start=(dp == 0),
                        stop=(dp == DPT - 1),
                    )
                mean = f32w.tile([1, 512], F32, tag="mean", bufs=1)
                nc.vector.tensor_scalar_mul(mean[0:1, :MC], pssum[0:1, :MC], 1.0 / D)
                var = f32w.tile([1, 512], F32, tag="var", bufs=1)
                # var = sumsq/D - mean^2
                nc.vector.tensor_mul(var[0:1, :MC], mean[0:1, :MC], mean[0:1, :MC])
                nc.vector.scalar_tensor_tensor(
                    out=var[0:1, :MC],
                    in0=pssq[0:1, :MC],
                    scalar=1.0 / D,
                    in1=var[0:1, :MC],
                    op0=ALU.mult,
                    op1=ALU.subtract,
                )
                std = f32w.tile([1, 512], F32, tag="std", bufs=1)
                nc.scalar.activation(std[0:1, :MC], var[0:1, :MC], AF.Sqrt, bias=eps_sb[0:1, 0:1])
                rstd = f32w.tile([1, 512], F32R, tag="rstd", bufs=1)
                nc.vector.reciprocal(rstd[0:1, :MC], std[0:1, :MC])
                # negmean_rstd = -mean * rstd
                nmr = f32w.tile([1, 512], F32R, tag="nmr", bufs=1)
                nc.vector.scalar_tensor_tensor(
                    out=nmr[0:1, :MC],
                    in0=mean[0:1, :MC],
                    scalar=-1.0,
                    in1=rstd[0:1, :MC],
                    op0=ALU.mult,
                    op1=ALU.mult,
                )
                psrs = psb.tile([P, 512], F32, tag="big")
                nc.tensor.matmul(
                    psrs[:, :MC],
                    lhsT=ones_row_f32[0:1, :],
                    rhs=rstd[0:1, :MC],
                    start=True,
                    stop=True,
                )
                psnm = psb.tile([P, 512], F32, tag="big")
                nc.tensor.matmul(
                    psnm[:, :MC],
                    lhsT=ones_row_f32[0:1, :],
                    rhs=nmr[0:1, :MC],
                    start=True,
                    stop=True,
                )
                for dp in range(DPT):
                    if apply_ln_affine:
                        tmp = f32w.tile([P, MC], F32, tag="lntmp", bufs=2)
                        nc.vector.scalar_tensor_tensor(
                            out=tmp[:, :],
                            in0=xu[:, dp, :],
                            scalar=1.0,
                            in1=psrs[:, :MC],
                            op0=ALU.mult,
                            op1=ALU.mult,
                        )
                        nc.vector.tensor_add(tmp[:, :], tmp[:, :], psnm[:, :MC])
                        nc.vector.tensor_scalar(
                            out=xn_dst(dp),
                            in0=tmp[:, :],
                            scalar1=ln_sb[:, ln_row, dp:dp + 1],
                            scalar2=ln_sb[:, ln_row + 1, dp:dp + 1],
                            op0=ALU.mult,
                            op1=ALU.add,
                        )
                    else:
                        tmp = f32w.tile([P, MC], F32, tag="lntmp", bufs=2)
                        nc.vector.tensor_mul(tmp[:, :], xu[:, dp, :], psrs[:, :MC])
                        nc.vector.tensor_add(xn_dst(dp), tmp[:, :], psnm[:, :MC])

            x1f = f32w.tile([P, DPT, MC], F32, tag="x1f")
            layernorm(x1u, lambda dp: x1f[:, dp, :], 0)
            x1n = act.tile([P, DPT, MC], BF16, tag="x1n", bufs=2)
            for dp in range(DPT):
                copy_out(evac_engine(dp), x1n[:, dp, :], x1f[:, dp, :])

            # ---- FFN (interleaved FFN1 -> relu -> FFN2 accumulation) ----
            pso = [psb.tile([P, 512], F32, tag="big", name=f"pso{_i}") for _i in range(DPT)]
            x2u = f32w.tile([P, DPT, MC], F32R, tag="x2u")
            for d1 in range(D1T):
                psf = psr.tile([P, 512], F32, tag="rows")
                for ft in range(FT):
                    nc.tensor.matmul(
                        psf[:, :MC],
                        lhsT=w1_sb[:, ft, d1 * P:(d1 + 1) * P],
                        rhs=x1n[:, ft, :],
                        start=(ft == 0),
                        stop=(ft == FT - 1),
                    )
                hrelu = act.tile([P, MC], BF16, tag="hrelu", bufs=6)
                if d1 % 2 == 0:
                    nc.vector.tensor_scalar(
                        out=hrelu[:, :],
                        in0=psf[:, :MC],
                        scalar1=b1c_sb[:, d1:d1 + 1],
                        scalar2=0.0,
                        op0=ALU.add,
                        op1=ALU.max,
                    )
                else:
                    nc.scalar.activation(
                        hrelu[:, :], psf[:, :MC], AF.Relu,
                        bias=b1c_sb[:, d1:d1 + 1],
                    )
                for dp in range(DPT):
                    nc.tensor.matmul(
                        pso[dp][:, :MC],
                        lhsT=w2_sb[:, d1, dp * P:(dp + 1) * P],
                        rhs=hrelu[:, :],
                        start=(d1 == 0),
                        stop=(d1 == D1T - 1),
                    )
            for dp in range(DPT):
                nc.vector.scalar_tensor_tensor(
                    out=x2u[:, dp, :],
                    in0=pso[dp][:, :MC],
                    scalar=b2c_sb[:, dp:dp + 1],
                    in1=x1f[:, dp, :],
                    op0=ALU.add,
                    op1=ALU.add,
                )

            x2n = f32w.tile([P, DPT, MC], F32, tag="x2n")
            layernorm(x2u, lambda dp: x2n[:, dp, :], 2)

            # ---- transpose back to row layout and store ----
            ccs = [(0, 128), (128, 128), (256, 64)]
            for cc, (c0, cw) in enumerate(ccs):
                osb = f32w.tile([P, F], F32, tag="osb", bufs=2)
                for dp in range(DPT):
                    pst = psr.tile([P, 512], F32, tag="rows")
                    nc.tensor.transpose(
                        pst[:cw, :P], x2n[:, dp, c0:c0 + cw], ident[:, :]
                    )
                    copy_out(
                        evac_engine(dp), osb[:cw, dp * P:(dp + 1) * P],
                        pst[:cw, :P],
                    )
                nc.sync.dma_start(
                    out_flat[m0 + c0:m0 + c0 + cw, :], osb[:cw, :]
                )

        _stack.close()

    nc.compile()
    return nc


def _prep_inputs(inputs):
    """Host-side weight fusion; returns per-core in_maps."""
    bf = ml_dtypes.bfloat16
    x = np.ascontiguousarray(inputs["x"], dtype=np.float32)
    Wq = inputs["Wq"].astype(np.float32)
    Wk = inputs["Wk"].astype(np.float32)
    Wv = inputs["Wv"].astype(np.float32)
    Wp = inputs["Wp"].astype(np.float32).reshape(H, D, D)
    sc = 1.0 / math.sqrt(D)
    wq_p = (Wq.transpose(1, 0, 2).reshape(F, H * D) * sc).astype(bf)
    wk_p = Wk.transpose(1, 0, 2).reshape(F, H * D).astype(bf)
    wvp_p = np.einsum("hfd,hde->hfe", Wv, Wp).reshape(H * F, D).astype(bf)
    cg_p = np.einsum("hd,hde->he", inputs["bv"].astype(np.float32), Wp).astype(bf)
    w1_p = inputs["W1"].astype(bf)
    w2_p = inputs["W2"].astype(bf)
    wg_p = (inputs["Wg"].astype(np.float32) / S).astype(bf)
    def col(v, nt):
        return np.ascontiguousarray(
            v.astype(np.float32).reshape(nt, 128).T
        )

    bqc_p = col(inputs["bq"].reshape(-1) * sc, TT)
    bkc_p = col(inputs["bk"].reshape(-1), TT)
    bpc_p = col(inputs["bp"], DPT)
    b1c_p = col(inputs["b1"], D1T)
    b2c_p = col(inputs["b2"], DPT)
    bg_p = inputs["bg"].astype(np.float32).reshape(1, H).astype(bf)
    ln_p = np.stack(
        [inputs["g1"], inputs["be1"], inputs["g2"], inputs["be2"]]
    ).astype(np.float32)
    apply_affine = not (
        np.all(ln_p[0] == 1) and np.all(ln_p[1] == 0)
        and np.all(ln_p[2] == 1) and np.all(ln_p[3] == 0)
    )
    shared = dict(
        wq_p=wq_p, wk_p=wk_p, wvp_p=wvp_p, cg_p=cg_p, w1_p=w1_p, w2_p=w2_p,
        wg_p=wg_p, bqc_p=bqc_p, bkc_p=bkc_p, bpc_p=bpc_p, b1c_p=b1c_p,
        b2c_p=b2c_p, bg_p=bg_p, mask_p=_make_mask(),
    )
    if apply_affine:
        shared["ln_p"] = ln_p
    x_bf = x.reshape(-1, F).astype(bf)
    in_maps = []
    for c in range(NCORES):
        m = dict(shared)
        m["x_bf"] = np.ascontiguousarray(x_bf[c * BC * S:(c + 1) * BC * S])
        in_maps.append(m)
    return in_maps, apply_affine


def _prep_inputs_small(inputs, nsamp):
    """Single map covering the first nsamp samples (for CoreSim tests)."""
    sub = dict(inputs)
    sub["x"] = np.asarray(inputs["x"])[:nsamp]
    maps, apply_affine = _prep_inputs(sub)
    m = maps[0]
    m["x_bf"] = m["x_bf"][: nsamp * S]
    return m, apply_affine


def _make_mask():
    m = np.zeros((120, 120), dtype=np.float32)
    for b in range(12):
        m[10 * b:10 * b + 10, 10 * b:10 * b + 10] = 1.0
    return m.astype(ml_dtypes.bfloat16)


_CACHED = {}


def _get_kernel(apply_affine):
    key = apply_affine
    if key not in _CACHED:
        _CACHED[key] = build_kernel(apply_affine)
    return _CACHED[key]


def kernel(**inputs):
    from concourse.bass_utils import run_bass_kernel_spmd

    in_maps, apply_affine = _prep_inputs(inputs)
    nc = _get_kernel(apply_affine)
    res = run_bass_kernel_spmd(nc, in_maps, list(range(NCORES)))
    outs = [np.asarray(r["out"]).reshape(BC, S, F) for r in res.results]
    return np.concatenate(outs, axis=0)


if __name__ == "__main__":
    nc = build_kernel(False)
    print("built ok")



# revision 3
# speedup vs baseline: 1.3746x; 1.3746x over previous
"""Trainium2 Bass kernel for nn_AttnBlock (dense transformer block), v2.

Strategy (pure data-parallel over batch, 8 cores):
  - Each core gets B/8 = 512 samples; all weights replicated.
  - Algebraic fusion (host-side, weights only):
      * K projection eliminated: scores = x (Wq Wk^T/sqrt(D)) x^T per head.
        Qm = x @ M with M_h = Wq_h Wk_h^T / sqrt(D) replaces BOTH Q and K.
        The bq-side bias term (Wk_h bq_h)·x_key is applied as a per-key
        additive bias on the exp (softmax-row-shift removes the bk terms).
      * attention applied to raw x:  y_h = attn_h @ x    (per sample)
      * V-projection and output projection fused: Wvp_h = Wv_h @ Wp_h
      * per-head bias folded via the gate vector: proj += gate @ (bv_h Wp_h)
      * gate/softmax normalization folded into the attention weights
        (pre-attnx), so PSUM evacuations are plain copies (DVE/Act split).
  - GPSIMD (Pool) used for partition-broadcasts and LN applies.
  - Output stored in T-layout [D, rows]; transposed on host.
  - 2-stage software pipeline: chunk n's FFN is interleaved into chunk
    n+1's front half to keep the PE busy across dependency gaps.

Self-contained: hardcodes shapes; imports only the concourse stack.
"""

import math
import os
import sys

import numpy as np

for _p in ("/opt/trn_rl_repo", os.path.expanduser("~/.axon_site/_ro/trn_rl_repo")):
    if os.path.isdir(_p) and _p not in sys.path:
        sys.path.insert(0, _p)

import ml_dtypes  # noqa: E402

import concourse.bass as bass  # noqa: E402
import concourse.mybir as mybir  # noqa: E402
import concourse.tile as tile  # noqa: E402
from concourse import bacc  # noqa: E402
from concourse.masks import make_identity  # noqa: E402

F32 = mybir.dt.float32
BF16 = mybir.dt.bfloat16
F32R = mybir.dt.float32r
AF = mybir.ActivationFunctionType
ALU = mybir.AluOpType

# Problem shapes (hardcoded per spec)
B, S, F, D, H = 4096, 10, 512, 512, 4
EPS = 1e-5
NCORES = 8
BC = B // NCORES          # samples per core = 512
P = 128

# Tiling
C = 32                    # samples per chunk
NCH = BC // C             # 16 chunks
MC = C * S                # 320 rows per chunk
WINS = (12, 12, 8)        # samples per attention window (sum = C)
FT = F // P               # 4 input-feature tiles
TT = (H * F) // P         # 16 Qm tiles (head-major over x-features)
D1T = (4 * D) // P        # 16 ffn hidden tiles
DPT = D // P              # 4 d_model tiles


def build_kernel(apply_ln_affine: bool, nch: int = NCH, debug: bool = False,
                 pipeline: bool = True):
    MR = nch * MC  # rows handled by this program
    nc = bacc.Bacc(None, target_bir_lowering=False, debug=debug)
    names = {}

    _lp = nc.allow_low_precision(reason="float32r intermediates are 4-byte")
    _lp.__enter__()
    with tile.TileContext(nc) as tc:
        with tc.tile_pool(name="dram", bufs=1, space="DRAM") as dram:
            x_bf = dram.tile([MR, F], BF16, kind="ExternalInput", name="x_bf", uniquify=False)
            m_d = dram.tile([F, H * F], BF16, kind="ExternalInput", name="m_p", uniquify=False)
            wvp_d = dram.tile([H * F, D], BF16, kind="ExternalInput", name="wvp_p", uniquify=False)
            w1_d = dram.tile([D, 4 * D], BF16, kind="ExternalInput", name="w1_p", uniquify=False)
            w2_d = dram.tile([4 * D, D], BF16, kind="ExternalInput", name="w2_p", uniquify=False)
            wg_d = dram.tile([F, H], BF16, kind="ExternalInput", name="wg_p", uniquify=False)
            cg_d = dram.tile([H, D], BF16, kind="ExternalInput", name="cg_p", uniquify=False)
            cq_d = dram.tile([F, H], BF16, kind="ExternalInput", name="cq_p", uniquify=False)
            bpc_d = dram.tile([P, DPT], F32, kind="ExternalInput", name="bpc_p", uniquify=False)
            b1c_d = dram.tile([P, D1T], F32, kind="ExternalInput", name="b1c_p", uniquify=False)
            b2c_d = dram.tile([P, DPT], F32, kind="ExternalInput", name="b2c_p", uniquify=False)
            bg_d = dram.tile([1, H], BF16, kind="ExternalInput", name="bg_p", uniquify=False)
            mask_d = dram.tile([120, 120], BF16, kind="ExternalInput", name="mask_p", uniquify=False)
            if apply_ln_affine:
                ln_d = dram.tile([4, D], F32, kind="ExternalInput", name="ln_p", uniquify=False)
            # output in T-layout: [D, rows]; transposed on host
            out_d = dram.tile([D, MR], F32, kind="ExternalOutput", name="out", uniquify=False)
        names["out"] = "out"

        from contextlib import ExitStack
        _stack = ExitStack()
        const = _stack.enter_context(tc.tile_pool(name="const", bufs=1))
        wts = _stack.enter_context(tc.tile_pool(name="wts", bufs=1))
        act = _stack.enter_context(tc.tile_pool(name="act", bufs=1))
        f32w = _stack.enter_context(tc.tile_pool(name="f32w", bufs=1))
        psq = _stack.enter_context(tc.tile_pool(name="psq", bufs=2, space="PSUM"))
        psb = _stack.enter_context(tc.tile_pool(name="psb", bufs=4, space="PSUM"))
        psr = _stack.enter_context(tc.tile_pool(name="psr", bufs=1, space="PSUM"))
        psy = _stack.enter_context(tc.tile_pool(name="psy", bufs=1, space="PSUM"))

        # ---- constants ----
        ident = const.tile([P, P], F32, tag="ident")
        make_identity(nc, ident)
        ones_row_bf = const.tile([1, 512], BF16, tag="ones_row_bf")
        nc.vector.memset(ones_row_bf[:], 1.0)
        ones_tmp = const.tile([P, P], F32, tag="ones_tmp")
        nc.vector.memset(ones_tmp[:], 1.0)
        ones_col_f32 = const.tile([P, 1], F32R, tag="ones_col_f32")
        nc.vector.tensor_copy(ones_col_f32[:], ones_tmp[:, 0:1])
        ones_col_bf = const.tile([P, 1], BF16, tag="ones_col_bf")
        nc.vector.memset(ones_col_bf[:], 1.0)
        eps_sb = const.tile([1, 1], F32, tag="eps")
        nc.vector.memset(eps_sb[:], EPS)
        # f32 whose bits are the rsqrt seed magic 0x5f3759df
        _magicf = float(np.frombuffer(
            np.uint32(0x5F3759DF).tobytes(), np.float32)[0])
        magic_sb = const.tile([1, 512], F32, tag="magic")
        nc.vector.memset(magic_sb[:], _magicf)
        neghalf_sb = const.tile([1, 512], F32, tag="neghalf")
        nc.vector.memset(neghalf_sb[:], -0.5)
        c15_sb = const.tile([1, 512], F32, tag="c15")
        nc.vector.memset(c15_sb[:], 1.5)
        mask_bd = const.tile([120, 120], BF16, tag="mask_bd")
        nc.sync.dma_start(mask_bd[:], mask_d[:])

        # ---- resident weights ----
        m_sb = wts.tile([P, FT, H * F], BF16, tag="m")
        wvp_sb = wts.tile([P, TT, D], BF16, tag="wvp")
        w1_sb = wts.tile([P, FT, 4 * D], BF16, tag="w1")
        w2_sb = wts.tile([P, D1T, D], BF16, tag="w2")
        wg_sb = wts.tile([P, FT, H], BF16, tag="wg")
        cg_sb = wts.tile([H, D], BF16, tag="cg")
        cq_sb = wts.tile([P, FT, H], BF16, tag="cq")
        bpc_sb = wts.tile([P, DPT], F32, tag="bpc")
        b1c_sb = wts.tile([P, D1T], F32, tag="b1c")
        b2c_sb = wts.tile([P, DPT], F32, tag="b2c")
        bg_sb = wts.tile([1, H], BF16, tag="bg")
        # small tensors first so early chunks aren't blocked behind big DMAs
        nc.sync.dma_start(bpc_sb[:], bpc_d[:])
        nc.sync.dma_start(b1c_sb[:], b1c_d[:])
        nc.sync.dma_start(b2c_sb[:], b2c_d[:])
        nc.sync.dma_start(bg_sb[:], bg_d[:])
        nc.sync.dma_start(cq_sb[:], cq_d[:].rearrange("(t p) n -> p t n", p=P))
        nc.sync.dma_start(wg_sb[:], wg_d[:].rearrange("(t p) n -> p t n", p=P))
        nc.sync.dma_start(cg_sb[:], cg_d[:])
        if apply_ln_affine:
            ln_sb = wts.tile([P, 4, DPT], F32, tag="ln")
            nc.sync.dma_start(ln_sb[:], ln_d[:].rearrange("r (t p) -> p r t", p=P))

        def load_big_weights(stage):
            # staged so chunk 0's compute starts after only m (+xt) arrive
            if stage == 0:
                nc.sync.dma_start(m_sb[:], m_d[:].rearrange("(t p) n -> p t n", p=P))
            elif stage == 1:
                nc.sync.dma_start(
                    wvp_sb[:], wvp_d[:].rearrange("(t p) n -> p t n", p=P))
            elif stage == 2:
                nc.sync.dma_start(w1_sb[:], w1_d[:].rearrange("(t p) n -> p t n", p=P))
                nc.sync.dma_start(w2_sb[:], w2_d[:].rearrange("(t p) n -> p t n", p=P))

        x_flat = x_bf[:]
        out_flat = out_d[:]

        # per-chunk state passed between segments (keyed by chunk index)
        st = {}

        def load_xt(ch):
            m0 = ch * MC
            xt = act.tile([P, FT, MC], BF16, tag="xt", bufs=3)
            for ft in range(FT):
                nc.sync.dma_start(
                    xt[:, ft, :],
                    x_flat[m0:m0 + MC, ft * P:(ft + 1) * P],
                    transpose=True,
                )
            st.setdefault(ch, {})["xt"] = xt

        def load_xw(ch):
            m0 = ch * MC
            xw = act.tile([120, len(WINS), F], BF16, tag="xw", bufs=3)
            wo = 0
            for w, wn in enumerate(WINS):
                nc.sync.dma_start(
                    xw[:wn * S, w, :], x_flat[m0 + wo * S:m0 + (wo + wn) * S, :]
                )
                wo += wn
            st.setdefault(ch, {})["xw"] = xw

        # ---------------- front half: Qm, gate, attention, Wvp, LN1 ----------
        def front_segments(ch):
            s = st[ch]
            segs = []

            # B: Qm projection, one segment per output tile
            qm = act.tile([P, TT, MC], BF16, tag="qm", bufs=1)
            s["qm"] = qm

            def mk_qm(t):
                def seg():
                    xt = s["xt"]
                    ps = psq.tile([P, 512], F32, tag="psq")
                    for ft in range(FT):
                        nc.tensor.matmul(
                            ps[:, :MC],
                            lhsT=m_sb[:, ft, t * P:(t + 1) * P],
                            rhs=xt[:, ft, :],
                            start=(ft == 0),
                            stop=(ft == FT - 1),
                        )
                    if t % 2 == 0:
                        nc.vector.tensor_copy(qm[:, t, :], ps[:, :MC])
                    else:
                        nc.scalar.copy(qm[:, t, :], ps[:, :MC])
                return seg
            qm_segs = [mk_qm(t) for t in range(TT)]
            segs.extend(qm_segs[:4])

            # C: gate = softmax(mean(x) @ Wg + bg), replicated over s
            def seg_gate():
                xt = s["xt"]
                xm_bf = act.tile([P, FT, C], BF16, tag="xm", bufs=2)
                for ft in range(FT):
                    nc.vector.tensor_reduce(
                        xm_bf[:, ft, :],
                        xt[:, ft, :].rearrange("p (b s) -> p b s", s=S),
                        axis=mybir.AxisListType.X,
                        op=ALU.add,
                    )
                psg = psy.tile([C, H], F32, tag="psy", name="psg")
                for ft in range(FT):
                    nc.tensor.matmul(
                        psg[:, :], lhsT=xm_bf[:, ft, :], rhs=wg_sb[:, ft, :],
                        start=(ft == 0), stop=False,
                    )
                nc.tensor.matmul(
                    psg[:, :], lhsT=ones_row_bf[0:1, :C], rhs=bg_sb[0:1, :],
                    start=False, stop=True,
                )
                eg = f32w.tile([C, H], F32, tag="eg", bufs=2)
                zg = f32w.tile([C, 1], F32, tag="zg", bufs=2)
                nc.scalar.activation(eg[:, :], psg[:, :], AF.Exp, accum_out=zg[:, :])
                rzg = f32w.tile([C, 1], F32, tag="rzg", bufs=2)
                nc.vector.reciprocal(rzg[:, :], zg[:, :])
                gatef = f32w.tile([C, H], F32, tag="gatef", bufs=2)
                nc.vector.tensor_scalar_mul(gatef[:, :], eg[:, :], rzg[:, :])
                psgt = psy.tile([H, C], F32, tag="psy", name="psgt")
                nc.tensor.transpose(psgt[:, :], gatef[:, :], ident[:C, :C])
                gft = f32w.tile([H, C], F32, tag="gft", bufs=2)
                nc.vector.tensor_copy(gft[:, :], psgt[:, :])
                grep = act.tile([H, MC], BF16, tag="grep", bufs=2)
                for sp in range(S):
                    nc.vector.tensor_copy(
                        grep[:, :].rearrange("h (b s) -> h b s", s=S)[:, :, sp],
                        gft[:, :],
                    )
                s["grep"] = grep
                # gate row gathered onto partition 0: [1, H*C] then
                # seq-replicated to [1, H*MC] (for the per-query 1/Z * gate row)
                growc = f32w.tile([1, H * C], F32, tag="growc", bufs=2)
                nc.sync.dma_start(growc[0:1, :], gft[:, :])
                growr = f32w.tile([1, H * MC], BF16, tag="growr", bufs=2)
                for sp in range(S):
                    nc.vector.tensor_copy(
                        growr[0:1, :].rearrange(
                            "o (h b s) -> o h b s", b=C, s=S)[:, :, :, sp],
                        growc[0:1, :].rearrange("o (h b) -> o h b", b=C),
                    )
                s["growr"] = growr


            # D: attention windows
            ysc = act.tile([P, TT, MC], BF16, tag="ysc", bufs=1)
            s["ysc"] = ysc
            wofs = []
            wo = 0
            for wn in WINS:
                wofs.append(wo)
                wo += wn

            def seg_dcorr():
                xt = s["xt"]
                for w in range(len(WINS)):
                    wn = WINS[w]
                    L = wn * S
                    woS = wofs[w] * S
                    psd = psr.tile([120, H], F32, tag="psr", name="psd")
                    for ft in range(FT):
                        nc.tensor.matmul(
                            psd[:L, :],
                            lhsT=xt[:, ft, woS:woS + L],
                            rhs=cq_sb[:, ft, :],
                            start=(ft == 0),
                            stop=(ft == FT - 1),
                        )
                    dsb = f32w.tile([120, H], F32, tag="dsb", bufs=4)
                    nc.scalar.copy(dsb[:L, :], psd[:L, :])
                    s[("dsb", w)] = dsb

            def seg_gate_dcorr():
                seg_gate()
                seg_dcorr()
            segs.append(seg_gate_dcorr)
            segs.extend(qm_segs[4:])

            def mk_attn_a(w):
                # scores + exp + mask for all heads
                def seg():
                    xt, qm_ = s["xt"], s["qm"]
                    wn = WINS[w]
                    L = wn * S
                    woS = wofs[w] * S
                    dsb = s[("dsb", w)]
                    abds = []
                    for h in range(H):
                        pss = psq.tile([P, 512], F32, tag="psq")
                        for dt in range(FT):
                            nc.tensor.matmul(
                                pss[:L, :L],
                                lhsT=xt[:, dt, woS:woS + L],
                                rhs=qm_[:, h * FT + dt, woS:woS + L],
                                start=(dt == 0),
                                stop=(dt == FT - 1),
                            )
                        es = act.tile([120, 128], BF16, tag="es", bufs=3)
                        nc.scalar.activation(
                            es[:L, :L], pss[:L, :L], AF.Exp,
                            bias=dsb[:L, h:h + 1],
                        )
                        abd = act.tile([120, 128], BF16, tag="abd", bufs=5)
                        nc.vector.tensor_mul(abd[:L, :L], es[:L, :L], mask_bd[:L, :L])
                        abds.append(abd)
                    s[("abds", w)] = abds
                return seg

            def mk_attn_b(w):
                # Z colsums, 1/Z, gate row, broadcast, scaled attn weights
                def seg():
                    wn = WINS[w]
                    L = wn * S
                    woS = wofs[w] * S
                    abds = s[("abds", w)]
                    psz = psr.tile([1, 512], F32, tag="psr")
                    for h in range(H):
                        nc.tensor.matmul(
                            psz[0:1, h * L:h * L + L],
                            lhsT=ones_col_bf[:L, 0:1],
                            rhs=abds[h][:L, :L],
                            start=True, stop=True,
                        )
                    rz = f32w.tile([1, 512], F32, tag="rz", bufs=2)
                    nc.vector.reciprocal(rz[0:1, :H * L], psz[0:1, :H * L])
                    wrow = f32w.tile([1, 512], F32, tag="wrow", bufs=2)
                    nc.vector.tensor_mul(
                        wrow[0:1, :H * L].rearrange("o (h m) -> o h m", m=L),
                        rz[0:1, :H * L].rearrange("o (h m) -> o h m", m=L),
                        s["growr"][0:1, :].rearrange(
                            "o (h m) -> o h m", m=MC)[:, :, woS:woS + L],
                    )
                    wbc = f32w.tile([P, 512], F32, tag="wbc", bufs=2)
                    for h in range(H):
                        nc.gpsimd.partition_broadcast(
                            wbc[:, h * L:h * L + L], wrow[0:1, h * L:h * L + L]
                        )
                    abscs = []
                    for h in range(H):
                        absc = act.tile([120, 128], BF16, tag="absc", bufs=6)
                        nc.vector.tensor_mul(
                            absc[:L, :L], abds[h][:L, :L], wbc[:L, h * L:h * L + L]
                        )
                        abscs.append(absc)
                    s[("abscs", w)] = abscs
                return seg

            def mk_attn_c(w, fts):
                # attnx for feature tiles in fts + evac into ysc
                def seg():
                    xw = s["xw"]
                    wn = WINS[w]
                    L = wn * S
                    woS = wofs[w] * S
                    abscs = s[("abscs", w)]
                    for ft in fts:
                        pyt = psy.tile([P, 512], F32, tag="psy")
                        for h in range(H):
                            nc.tensor.matmul(
                                pyt[:, h * L:h * L + L],
                                lhsT=xw[:L, w, ft * P:(ft + 1) * P],
                                rhs=abscs[h][:L, :L],
                                start=True, stop=True,
                            )
                        dst = ysc[:, :, woS:woS + L].rearrange(
                            "p (h f) m -> p h f m", f=FT)[:, :, ft, :]
                        src = pyt[:, :H * L].rearrange("p (h m) -> p h m", m=L)
                        if ft % 2 == 0:
                            nc.vector.tensor_copy(dst, src)
                        else:
                            nc.scalar.copy(dst, src)
                return seg

            for w in range(len(WINS)):
                segs.append(mk_attn_a(w))
                segs.append(mk_attn_b(w))
                segs.append(mk_attn_c(w, (0, 1)))
                segs.append(mk_attn_c(w, (2, 3)))

            # E: fused projection + residual (LN1 stats deferred one dp so
            # the PE never waits on the stt/sq of the dp it just produced)
            x1u = f32w.tile([P, DPT, MC], F32R, tag="x1u", bufs=1)
            sq1 = f32w.tile([P, DPT, MC], F32R, tag="sq1", bufs=1)
            s["x1u"], s["sq1"] = x1u, sq1

            def stats1(dp):
                nc.tensor.matmul(
                    s["pstat1"][0:1, :MC],
                    lhsT=ones_col_f32[:, 0:1],
                    rhs=x1u[:, dp, :],
                    start=(dp == 0), stop=(dp == DPT - 1),
                )
                nc.tensor.matmul(
                    s["psqs1"][0:1, :MC],
                    lhsT=ones_col_f32[:, 0:1],
                    rhs=sq1[:, dp, :],
                    start=(dp == 0), stop=(dp == DPT - 1),
                )

            def mk_proj(dp):
                def seg():
                    if dp == 0:
                        # allocated here (not at construction) so the psr
                        # rotation order matches emission order
                        s["pstat1"] = psr.tile([1, 512], F32, tag="psr", name="pstat1")
                        s["psqs1"] = psy.tile([1, 512], F32, tag="psy", name="psqs1")
                    xt = s["xt"]
                    ps = psb.tile([P, 512], F32, tag="psb")
                    for t in range(TT):
                        nc.tensor.matmul(
                            ps[:, :MC],
                            lhsT=wvp_sb[:, t, dp * P:(dp + 1) * P],
                            rhs=ysc[:, t, :],
                            start=(t == 0), stop=False,
                        )
                    nc.tensor.matmul(
                        ps[:, :MC],
                        lhsT=cg_sb[:, dp * P:(dp + 1) * P],
                        rhs=s["grep"][:, :],
                        start=False, stop=True,
                    )
                    nc.vector.scalar_tensor_tensor(
                        out=x1u[:, dp, :],
                        in0=ps[:, :MC],
                        scalar=bpc_sb[:, dp:dp + 1],
                        in1=xt[:, dp, :],
                        op0=ALU.add,
                        op1=ALU.add,
                    )
                    nc.scalar.activation(sq1[:, dp, :], x1u[:, dp, :], AF.Square)
                    if dp > 0:
                        stats1(dp - 1)
                return seg
            for dp in range(DPT):
                segs.append(mk_proj(dp))
            return segs

        # LN1 of chunk ch, emitted early in iteration ch+1 (3-stage pipeline)
        def mid_segments(ch):
            s = st[ch]
            x1n = act.tile([P, DPT, MC], BF16, tag="x1n", bufs=1)
            s["x1n"] = x1n

            def seg_e_tail():
                dp = DPT - 1
                nc.tensor.matmul(
                    s["pstat1"][0:1, :MC],
                    lhsT=ones_col_f32[:, 0:1],
                    rhs=s["x1u"][:, dp, :],
                    start=(dp == 0), stop=(dp == DPT - 1),
                )
                nc.tensor.matmul(
                    s["psqs1"][0:1, :MC],
                    lhsT=ones_col_f32[:, 0:1],
                    rhs=s["sq1"][:, dp, :],
                    start=(dp == 0), stop=(dp == DPT - 1),
                )

            def seg_ln1_chain():
                rs_bc, nm_bc = ln_chain(s["pstat1"], s["psqs1"], tag="1")
                s["rs1"], s["nm1"] = rs_bc, nm_bc

            def seg_ln1_apply():
                ln_apply(s["x1u"], s["rs1"], s["nm1"],
                         lambda dp: x1n[:, dp, :], 0)
            return [seg_e_tail, seg_ln1_chain, seg_ln1_apply]

        # shared LN helpers (T-layout; stats already in pstat rows {0, 32}).
        # rstd = rsqrt(var) via the quake bit trick + 2 Newton steps, all on
        # GPSIMD so the Act engine never leaves the exp table set (eps is
        # dropped: row variance here is O(1), so eps=1e-5 is far below the
        # bf16 noise floor).
        def ln_chain(pstat, psqs, tag):
            U32 = mybir.dt.uint32
            # negmean = -sum/D (sign is irrelevant for the square)
            mean = f32w.tile([1, 512], F32, tag="mean", bufs=1)
            nc.vector.tensor_scalar_mul(mean[0:1, :MC], pstat[0:1, :MC], -1.0 / D)
            msq = f32w.tile([1, 512], F32, tag="msq", bufs=1)
            nc.scalar.activation(msq[0:1, :MC], mean[0:1, :MC], AF.Square)
            var = f32w.tile([1, 512], F32, tag="var", bufs=1)
            nc.vector.scalar_tensor_tensor(
                out=var[0:1, :MC], in0=psqs[0:1, :MC], scalar=1.0 / D,
                in1=msq[0:1, :MC], op0=ALU.mult, op1=ALU.subtract,
            )
            # rsqrt(var): quake seed (DVE shift/sub) + 2 Newton steps on
            # GPSIMD using mul/add TensorTensor only (ISA-legal on Pool)
            y0 = f32w.tile([1, 512], F32, tag="y0", bufs=1)
            nc.vector.tensor_scalar(
                out=y0[0:1, :MC].bitcast(U32), in0=var[0:1, :MC].bitcast(U32),
                scalar1=1, scalar2=None, op0=ALU.logical_shift_right,
            )
            nc.vector.tensor_tensor(
                out=y0[0:1, :MC].bitcast(U32),
                in0=magic_sb[0:1, :MC].bitcast(U32),
                in1=y0[0:1, :MC].bitcast(U32), op=ALU.subtract,
            )
            varh = f32w.tile([1, 512], F32, tag="varh", bufs=1)
            nc.gpsimd.tensor_mul(varh[0:1, :MC], var[0:1, :MC], neghalf_sb[0:1, :MC])
            rs_row = y0
            for _ in range(2):
                aa = f32w.tile([1, 512], F32, tag="aa", bufs=2)
                nc.gpsimd.tensor_mul(aa[0:1, :MC], rs_row[0:1, :MC], rs_row[0:1, :MC])
                nc.gpsimd.tensor_mul(aa[0:1, :MC], varh[0:1, :MC], aa[0:1, :MC])
                nc.gpsimd.tensor_add(aa[0:1, :MC], aa[0:1, :MC], c15_sb[0:1, :MC])
                yn = f32w.tile([1, 512], F32, tag="yn", bufs=2)
                nc.gpsimd.tensor_mul(yn[0:1, :MC], rs_row[0:1, :MC], aa[0:1, :MC])
                rs_row = yn
            # nm = -mean*rstd = negmean*rstd
            nm_row = f32w.tile([1, 512], F32, tag="nmr", bufs=1)
            nc.gpsimd.tensor_mul(nm_row[0:1, :MC], mean[0:1, :MC], rs_row[0:1, :MC])
            rs_bc = f32w.tile([P, MC], F32, tag="rsb", bufs=1)
            nc.gpsimd.partition_broadcast(rs_bc[:, :], rs_row[0:1, :MC])
            nm_bc = f32w.tile([P, MC], F32, tag="nmb", bufs=1)
            nc.gpsimd.partition_broadcast(nm_bc[:, :], nm_row[0:1, :MC])
            return rs_bc, nm_bc

        def ln_apply(xu, rs_bc, nm_bc, dst, ln_row):
            for dp in range(DPT):
                tmp = f32w.tile([P, MC], F32R, tag="lntmp", bufs=2)
                nc.vector.tensor_mul(tmp[:, :], xu[:, dp, :], rs_bc[:, :])
                if apply_ln_affine:
                    t2 = f32w.tile([P, MC], F32R, tag="lnt2", bufs=2)
                    nc.gpsimd.tensor_add(t2[:, :], tmp[:, :], nm_bc[:, :])
                    nc.vector.tensor_scalar(
                        out=dst(dp), in0=t2[:, :],
                        scalar1=ln_sb[:, ln_row, dp:dp + 1],
                        scalar2=ln_sb[:, ln_row + 1, dp:dp + 1],
                        op0=ALU.mult, op1=ALU.add,
                    )
                else:
                    nc.gpsimd.tensor_add(dst(dp), tmp[:, :], nm_bc[:, :])

        # ---------------- back half: FFN, LN2, store -------------------------
        def back_segments(ch):
            s = st[ch]
            m0 = ch * MC
            segs = []
            pso = [psb.tile([P, 512], F32, tag="psb", name=f"pso{ch}_{i}")
                   for i in range(DPT)]
            s["pso"] = pso

            def ffn1_half(d1):
                x1n = s["x1n"]
                psf = psq.tile([P, 512], F32, tag="psq")
                for ft in range(FT):
                    nc.tensor.matmul(
                        psf[:, :MC],
                        lhsT=w1_sb[:, ft, d1 * P:(d1 + 1) * P],
                        rhs=x1n[:, ft, :],
                        start=(ft == 0),
                        stop=(ft == FT - 1),
                    )
                hrelu = act.tile([P, MC], BF16, tag="hrelu", bufs=6)
                if d1 % 2 == 0:
                    nc.vector.tensor_scalar(
                        out=hrelu[:, :], in0=psf[:, :MC],
                        scalar1=b1c_sb[:, d1:d1 + 1], scalar2=0.0,
                        op0=ALU.add, op1=ALU.max,
                    )
                else:
                    nc.scalar.activation(
                        hrelu[:, :], psf[:, :MC], AF.Relu,
                        bias=b1c_sb[:, d1:d1 + 1],
                    )
                return hrelu

            def ffn2_half(d1, hrelu):
                for dp in range(DPT):
                    nc.tensor.matmul(
                        pso[dp][:, :MC],
                        lhsT=w2_sb[:, d1, dp * P:(dp + 1) * P],
                        rhs=hrelu[:, :],
                        start=(d1 == 0),
                        stop=(d1 == D1T - 1),
                    )

            def mk_ffn_pair(d1):
                def seg():
                    ha = ffn1_half(d1)
                    hb = ffn1_half(d1 + 1)
                    ffn2_half(d1, ha)
                    ffn2_half(d1 + 1, hb)
                return seg
            for d1 in range(0, D1T, 2):
                segs.append(mk_ffn_pair(d1))

            # H1: residual + LN2 stats
            x2u = f32w.tile([P, DPT, MC], F32R, tag="x2u", bufs=1)
            sq2 = f32w.tile([P, DPT, MC], F32R, tag="sq2", bufs=1)
            x2n = f32w.tile([P, DPT, MC], F32, tag="x2n", bufs=1)

            def seg_h1a():
                x1n = s["x1n"]
                for dp in range(DPT):
                    nc.vector.scalar_tensor_tensor(
                        out=x2u[:, dp, :],
                        in0=pso[dp][:, :MC],
                        scalar=b2c_sb[:, dp:dp + 1],
                        in1=x1n[:, dp, :],
                        op0=ALU.add,
                        op1=ALU.add,
                    )
                    nc.scalar.activation(sq2[:, dp, :], x2u[:, dp, :], AF.Square)
            segs.append(seg_h1a)

            def seg_h1b():
                pstat2 = psr.tile([1, 512], F32, tag="psr", name="pstat2")
                psqs2 = psy.tile([1, 512], F32, tag="psy", name="psqs2")
                s["pstat2"], s["psqs2"] = pstat2, psqs2
                for dp in range(DPT):
                    nc.tensor.matmul(
                        pstat2[0:1, :MC],
                        lhsT=ones_col_f32[:, 0:1],
                        rhs=x2u[:, dp, :],
                        start=(dp == 0), stop=(dp == DPT - 1),
                    )
                    nc.tensor.matmul(
                        psqs2[0:1, :MC],
                        lhsT=ones_col_f32[:, 0:1],
                        rhs=sq2[:, dp, :],
                        start=(dp == 0), stop=(dp == DPT - 1),
                    )
            segs.append(seg_h1b)

            def seg_h2():
                rs_bc, nm_bc = ln_chain(s["pstat2"], s["psqs2"], tag="2")
                s["rs2"], s["nm2"] = rs_bc, nm_bc
            segs.append(seg_h2)

            def seg_h3():
                ln_apply(x2u, s["rs2"], s["nm2"],
                         lambda dp: x2n[:, dp, :], 2)
            segs.append(seg_h3)

            def seg_store():
                for dp in range(DPT):
                    nc.sync.dma_start(
                        out_flat[dp * P:(dp + 1) * P, m0:m0 + MC],
                        x2n[:, dp, :],
                    )
                st.pop(ch, None)
            segs.append(seg_store)
            return segs

        # ---------------- emission: 3-stage software pipeline ----------------
        # front seg indices: B=0..15, C=16, D=17..28 (4 per window), E=29..32
        # mid (ch-1): 0=E-tail stats, 1=LN1 chain, 2=LN1 apply
        # back (ch-1): G=0..15 (ffn d1), H1=16, H2=17, H3=18, I=19
        insert_after = {
            2: [("m", 0)],                              # LN1 dp3 stats
            3: [("m", 1)],                              # LN1 chain
            11: [("m", 2)],                             # LN1 apply -> x1n
            16: [("b", 0)], 17: [("b", 1)], 18: [("b", 2)], 19: [("b", 3)],
            20: [("b", 4)], 21: [("b", 5)], 22: [("b", 6)], 23: [("b", 7)],
            24: [("b", 8)],                             # H1a: x2u evac (DVE)
            25: [("b", 9)],                             # H1b: LN2 stats
            26: [("b", 10)],                            # H2: LN2 chain
            28: [("b", 11)],                            # H3: LN2 apply
            29: [("b", 12)],                            # store
        }

        load_xt(0)
        load_big_weights(0)      # m
        load_xw(0)
        load_big_weights(1)      # wvp
        if nch > 1:
            load_xt(1)
            load_xw(1)
        load_big_weights(2)      # w1, w2
        prev_back = None
        prev_ch = None
        for ch in range(nch):
            fsegs = front_segments(ch)
            mids = mid_segments(prev_ch) if prev_ch is not None else None
            if pipeline and prev_back is not None:
                done = set()
                for fi, fseg in enumerate(fsegs):
                    fseg()
                    for kind, bi in insert_after.get(fi, ()):
                        (mids if kind == "m" else prev_back)[bi]()
                        done.add((kind, bi))
                for bi in range(len(mids)):
                    if ("m", bi) not in done:
                        mids[bi]()
                for bi in range(len(prev_back)):
                    if ("b", bi) not in done:
                        prev_back[bi]()
            else:
                if mids is not None:
                    for mseg in mids:
                        mseg()
                if prev_back is not None:
                    for bseg in prev_back:
                        bseg()
                for fseg in fsegs:
                    fseg()
            if ch + 2 < nch:
                load_xt(ch + 2)
                load_xw(ch + 2)
            prev_back = back_segments(ch)
            prev_ch = ch
        for mseg in mid_segments(prev_ch):
            mseg()
        for bseg in prev_back:
            bseg()

        _stack.close()

    nc.compile()
    return nc


def _prep_inputs(inputs):
    """Host-side weight fusion; returns per-core in_maps."""
    bf = ml_dtypes.bfloat16
    x = np.ascontiguousarray(inputs["x"], dtype=np.float32)
    Wq = inputs["Wq"].astype(np.float32)
    Wk = inputs["Wk"].astype(np.float32)
    Wv = inputs["Wv"].astype(np.float32)
    Wp = inputs["Wp"].astype(np.float32).reshape(H, D, D)
    sc = 1.0 / math.sqrt(D)
    # M_h = Wq_h Wk_h^T / sqrt(D), stacked head-major on columns: [F, H*F]
    M = np.einsum("hfd,hgd->hfg", Wq, Wk) * sc
    m_p = np.ascontiguousarray(M.transpose(1, 0, 2).reshape(F, H * F)).astype(bf)
    # c_h = Wk_h bq_h / sqrt(D): per-key additive bias -> [F, H]
    cq_p = (np.einsum("hfd,hd->hf", Wk, inputs["bq"].astype(np.float32))
            * sc).T.astype(bf)
    cq_p = np.ascontiguousarray(cq_p)
    wvp_p = np.einsum("hfd,hde->hfe", Wv, Wp).reshape(H * F, D).astype(bf)
    cg_p = np.einsum("hd,hde->he", inputs["bv"].astype(np.float32), Wp).astype(bf)
    w1_p = inputs["W1"].astype(bf)
    w2_p = inputs["W2"].astype(bf)
    wg_p = (inputs["Wg"].astype(np.float32) / S).astype(bf)

    def col(v, nt):
        return np.ascontiguousarray(v.astype(np.float32).reshape(nt, 128).T)

    bpc_p = col(inputs["bp"], DPT)
    b1c_p = col(inputs["b1"], D1T)
    b2c_p = col(inputs["b2"], DPT)
    bg_p = inputs["bg"].astype(np.float32).reshape(1, H).astype(bf)
    ln_p = np.stack(
        [inputs["g1"], inputs["be1"], inputs["g2"], inputs["be2"]]
    ).astype(np.float32)
    apply_affine = not (
        np.all(ln_p[0] == 1) and np.all(ln_p[1] == 0)
        and np.all(ln_p[2] == 1) and np.all(ln_p[3] == 0)
    )
    shared = dict(
        m_p=m_p, cq_p=cq_p, wvp_p=wvp_p, cg_p=cg_p, w1_p=w1_p, w2_p=w2_p,
        wg_p=wg_p, bpc_p=bpc_p, b1c_p=b1c_p, b2c_p=b2c_p, bg_p=bg_p,
        mask_p=_make_mask(),
    )
    if apply_affine:
        shared["ln_p"] = ln_p
    x_bf = x.reshape(-1, F).astype(bf)
    in_maps = []
    for c in range(NCORES):
        m = dict(shared)
        m["x_bf"] = np.ascontiguousarray(x_bf[c * BC * S:(c + 1) * BC * S])
        in_maps.append(m)
    return in_maps, apply_affine


def _prep_inputs_small(inputs, nsamp):
    """Single map covering the first nsamp samples (for CoreSim tests)."""
    sub = dict(inputs)
    sub["x"] = np.asarray(inputs["x"])[:nsamp]
    maps, apply_affine = _prep_inputs(sub)
    m = maps[0]
    m["x_bf"] = m["x_bf"][: nsamp * S]
    return m, apply_affine


def _make_mask():
    m = np.zeros((120, 120), dtype=np.float32)
    for b in range(12):
        m[10 * b:10 * b + 10, 10 * b:10 * b + 10] = 1.0
    return m.astype(ml_dtypes.bfloat16)


_CACHED = {}


def _get_kernel(apply_affine):
    key = apply_affine
    if key not in _CACHED:
        _CACHED[key] = build_kernel(apply_affine)
    return _CACHED[key]


def kernel(**inputs):
    from concourse.bass_utils import run_bass_kernel_spmd

    in_maps, apply_affine = _prep_inputs(inputs)
    nc = _get_kernel(apply_affine)
    res = run_bass_kernel_spmd(nc, in_maps, list(range(NCORES)))
    outs = [
        np.asarray(r["out"]).reshape(D, BC * S).T.reshape(BC, S, F)
        for r in res.results
    ]
    return np.concatenate(outs, axis=0)


if __name__ == "__main__":
    nc = build_kernel(False)
    print("built ok")


# revision 4
# speedup vs baseline: 1.4412x; 1.0484x over previous
"""Trainium2 Bass kernel for nn_AttnBlock (dense transformer block), v2.

Strategy (pure data-parallel over batch, 8 cores):
  - Each core gets B/8 = 512 samples; all weights replicated.
  - Algebraic fusion (host-side, weights only):
      * K projection eliminated: scores = x (Wq Wk^T/sqrt(D)) x^T per head.
        Qm = x @ M with M_h = Wq_h Wk_h^T / sqrt(D) replaces BOTH Q and K.
        The bq-side bias term (Wk_h bq_h)·x_key is applied as a per-key
        additive bias on the exp (softmax-row-shift removes the bk terms).
      * attention applied to raw x:  y_h = attn_h @ x    (per sample)
      * V-projection and output projection fused: Wvp_h = Wv_h @ Wp_h
      * per-head bias folded via the gate vector: proj += gate @ (bv_h Wp_h)
      * gate/softmax normalization folded into the attention weights
        (pre-attnx), so PSUM evacuations are plain copies (DVE/Act split).
  - GPSIMD (Pool) used for partition-broadcasts and LN applies.
  - Output stored in T-layout [D, rows]; transposed on host.
  - 2-stage software pipeline: chunk n's FFN is interleaved into chunk
    n+1's front half to keep the PE busy across dependency gaps.

Self-contained: hardcodes shapes; imports only the concourse stack.
"""

import math
import os
import sys

import numpy as np

for _p in ("/opt/trn_rl_repo", os.path.expanduser("~/.axon_site/_ro/trn_rl_repo")):
    if os.path.isdir(_p) and _p not in sys.path:
        sys.path.insert(0, _p)

import ml_dtypes  # noqa: E402

import concourse.bass as bass  # noqa: E402
import concourse.mybir as mybir  # noqa: E402
import concourse.tile as tile  # noqa: E402
from concourse import bacc  # noqa: E402
from concourse.masks import make_identity  # noqa: E402

F32 = mybir.dt.float32
BF16 = mybir.dt.bfloat16
F32R = mybir.dt.float32r
AF = mybir.ActivationFunctionType
ALU = mybir.AluOpType

# Problem shapes (hardcoded per spec)
B, S, F, D, H = 4096, 10, 512, 512, 4
EPS = 1e-5
NCORES = 8
BC = B // NCORES          # samples per core = 512
P = 128

# Tiling
C = 32                    # samples per chunk
NCH = BC // C             # 16 chunks
MC = C * S                # 320 rows per chunk
WINS = (12, 12, 8)        # samples per attention window (sum = C)
FT = F // P               # 4 input-feature tiles
TT = (H * F) // P         # 16 Qm tiles (head-major over x-features)
D1T = (4 * D) // P        # 16 ffn hidden tiles
DPT = D // P              # 4 d_model tiles


def build_kernel(apply_ln_affine: bool, nch: int = NCH, debug: bool = False,
                 pipeline: bool = True):
    MR = nch * MC  # rows handled by this program
    nc = bacc.Bacc(None, target_bir_lowering=False, debug=debug)
    names = {}

    _lp = nc.allow_low_precision(reason="float32r intermediates are 4-byte")
    _lp.__enter__()
    with tile.TileContext(nc) as tc:
        with tc.tile_pool(name="dram", bufs=1, space="DRAM") as dram:
            x_bf = dram.tile([MR, F], BF16, kind="ExternalInput", name="x_bf", uniquify=False)
            m_d = dram.tile([F, H * F], BF16, kind="ExternalInput", name="m_p", uniquify=False)
            wvp_d = dram.tile([H * F, D], BF16, kind="ExternalInput", name="wvp_p", uniquify=False)
            w1_d = dram.tile([D, 4 * D], BF16, kind="ExternalInput", name="w1_p", uniquify=False)
            w2_d = dram.tile([4 * D, D], BF16, kind="ExternalInput", name="w2_p", uniquify=False)
            wg_d = dram.tile([F, H], BF16, kind="ExternalInput", name="wg_p", uniquify=False)
            cg_d = dram.tile([H, D], BF16, kind="ExternalInput", name="cg_p", uniquify=False)
            cq_d = dram.tile([F, H], BF16, kind="ExternalInput", name="cq_p", uniquify=False)
            bpc_d = dram.tile([P, DPT], F32, kind="ExternalInput", name="bpc_p", uniquify=False)
            b1c_d = dram.tile([P, D1T], F32, kind="ExternalInput", name="b1c_p", uniquify=False)
            b2c_d = dram.tile([P, DPT], F32, kind="ExternalInput", name="b2c_p", uniquify=False)
            bg_d = dram.tile([1, H], BF16, kind="ExternalInput", name="bg_p", uniquify=False)
            mask_d = dram.tile([120, 120], BF16, kind="ExternalInput", name="mask_p", uniquify=False)
            if apply_ln_affine:
                ln_d = dram.tile([4, D], F32, kind="ExternalInput", name="ln_p", uniquify=False)
            # output in T-layout: [D, rows]; transposed on host
            out_d = dram.tile([D, MR], F32, kind="ExternalOutput", name="out", uniquify=False)
        names["out"] = "out"

        from contextlib import ExitStack
        _stack = ExitStack()
        const = _stack.enter_context(tc.tile_pool(name="const", bufs=1))
        wts = _stack.enter_context(tc.tile_pool(name="wts", bufs=1))
        act = _stack.enter_context(tc.tile_pool(name="act", bufs=1))
        f32w = _stack.enter_context(tc.tile_pool(name="f32w", bufs=1))
        psq = _stack.enter_context(tc.tile_pool(name="psq", bufs=2, space="PSUM"))
        psb = _stack.enter_context(tc.tile_pool(name="psb", bufs=4, space="PSUM"))
        psr = _stack.enter_context(tc.tile_pool(name="psr", bufs=1, space="PSUM"))
        psy = _stack.enter_context(tc.tile_pool(name="psy", bufs=1, space="PSUM"))

        # ---- constants ----
        ident = const.tile([P, P], F32, tag="ident")
        make_identity(nc, ident)
        ones_row_bf = const.tile([1, 512], BF16, tag="ones_row_bf")
        nc.vector.memset(ones_row_bf[:], 1.0)
        ones_tmp = const.tile([P, P], F32, tag="ones_tmp")
        nc.vector.memset(ones_tmp[:], 1.0)
        ones_col_f32 = const.tile([P, 1], F32R, tag="ones_col_f32")
        nc.vector.tensor_copy(ones_col_f32[:], ones_tmp[:, 0:1])
        ones_col_bf = const.tile([P, 1], BF16, tag="ones_col_bf")
        nc.vector.memset(ones_col_bf[:], 1.0)
        eps_sb = const.tile([1, 1], F32, tag="eps")
        nc.vector.memset(eps_sb[:], EPS)
        # f32 whose bits are the rsqrt seed magic 0x5f3759df
        _magicf = float(np.frombuffer(
            np.uint32(0x5F3759DF).tobytes(), np.float32)[0])
        magic_sb = const.tile([1, 512], F32, tag="magic")
        nc.vector.memset(magic_sb[:], _magicf)
        neghalf_sb = const.tile([1, 512], F32, tag="neghalf")
        nc.vector.memset(neghalf_sb[:], -0.5)
        c15_sb = const.tile([1, 512], F32, tag="c15")
        nc.vector.memset(c15_sb[:], 1.5)
        mask_bd = const.tile([120, 120], BF16, tag="mask_bd")
        nc.sync.dma_start(mask_bd[:], mask_d[:])

        # ---- resident weights ----
        m_sb = wts.tile([P, FT, H * F], BF16, tag="m")
        wvp_sb = wts.tile([P, TT, D], BF16, tag="wvp")
        w1_sb = wts.tile([P, FT, 4 * D], BF16, tag="w1")
        w2_sb = wts.tile([P, D1T, D], BF16, tag="w2")
        wg_sb = wts.tile([P, FT, H], BF16, tag="wg")
        cg_sb = wts.tile([H, D], BF16, tag="cg")
        cq_sb = wts.tile([P, FT, H], BF16, tag="cq")
        bpc_sb = wts.tile([P, DPT], F32, tag="bpc")
        b1c_sb = wts.tile([P, D1T], F32, tag="b1c")
        b2c_sb = wts.tile([P, DPT], F32, tag="b2c")
        bg_sb = wts.tile([1, H], BF16, tag="bg")
        # small tensors first so early chunks aren't blocked behind big DMAs
        nc.sync.dma_start(bpc_sb[:], bpc_d[:])
        nc.sync.dma_start(b1c_sb[:], b1c_d[:])
        nc.sync.dma_start(b2c_sb[:], b2c_d[:])
        nc.sync.dma_start(bg_sb[:], bg_d[:])
        nc.sync.dma_start(cq_sb[:], cq_d[:].rearrange("(t p) n -> p t n", p=P))
        nc.sync.dma_start(wg_sb[:], wg_d[:].rearrange("(t p) n -> p t n", p=P))
        nc.sync.dma_start(cg_sb[:], cg_d[:])
        if apply_ln_affine:
            ln_sb = wts.tile([P, 4, DPT], F32, tag="ln")
            nc.sync.dma_start(ln_sb[:], ln_d[:].rearrange("r (t p) -> p r t", p=P))

        def load_big_weights(stage):
            # staged so chunk 0's compute starts after only m (+xt) arrive
            if stage == 0:
                nc.sync.dma_start(m_sb[:], m_d[:].rearrange("(t p) n -> p t n", p=P))
            elif stage == 1:
                nc.sync.dma_start(
                    wvp_sb[:], wvp_d[:].rearrange("(t p) n -> p t n", p=P))
            elif stage == 2:
                nc.sync.dma_start(w1_sb[:], w1_d[:].rearrange("(t p) n -> p t n", p=P))
                nc.sync.dma_start(w2_sb[:], w2_d[:].rearrange("(t p) n -> p t n", p=P))

        x_flat = x_bf[:]
        out_flat = out_d[:]

        # per-chunk state passed between segments (keyed by chunk index)
        st = {}

        def load_xt(ch):
            m0 = ch * MC
            xt = act.tile([P, FT, MC], BF16, tag="xt", bufs=3)
            for ft in range(FT):
                nc.sync.dma_start(
                    xt[:, ft, :],
                    x_flat[m0:m0 + MC, ft * P:(ft + 1) * P],
                    transpose=True,
                )
            st.setdefault(ch, {})["xt"] = xt

        def load_xw(ch):
            m0 = ch * MC
            xw = act.tile([120, len(WINS), F], BF16, tag="xw", bufs=3)
            wo = 0
            for w, wn in enumerate(WINS):
                nc.sync.dma_start(
                    xw[:wn * S, w, :], x_flat[m0 + wo * S:m0 + (wo + wn) * S, :]
                )
                wo += wn
            st.setdefault(ch, {})["xw"] = xw

        # ---------------- front half: Qm, gate, attention, Wvp, LN1 ----------
        def front_segments(ch):
            s = st[ch]
            segs = []

            # B: Qm projection, one segment per output tile
            qm = act.tile([P, TT, MC], BF16, tag="qm", bufs=1)
            s["qm"] = qm

            def mk_qm(t):
                def seg():
                    xt = s["xt"]
                    ps = psq.tile([P, 512], F32, tag="psq")
                    for ft in range(FT):
                        nc.tensor.matmul(
                            ps[:, :MC],
                            lhsT=m_sb[:, ft, t * P:(t + 1) * P],
                            rhs=xt[:, ft, :],
                            start=(ft == 0),
                            stop=(ft == FT - 1),
                        )
                    if t % 2 == 0:
                        nc.vector.tensor_copy(qm[:, t, :], ps[:, :MC])
                    else:
                        nc.scalar.copy(qm[:, t, :], ps[:, :MC])
                return seg
            qm_segs = [mk_qm(t) for t in range(TT)]
            segs.extend(qm_segs[:4])

            # C: gate = softmax(mean(x) @ Wg + bg), replicated over s
            def seg_gate():
                xt = s["xt"]
                xm_bf = act.tile([P, FT, C], BF16, tag="xm", bufs=2)
                for ft in range(FT):
                    nc.vector.tensor_reduce(
                        xm_bf[:, ft, :],
                        xt[:, ft, :].rearrange("p (b s) -> p b s", s=S),
                        axis=mybir.AxisListType.X,
                        op=ALU.add,
                    )
                psg = psy.tile([C, H], F32, tag="psy", name="psg")
                for ft in range(FT):
                    nc.tensor.matmul(
                        psg[:, :], lhsT=xm_bf[:, ft, :], rhs=wg_sb[:, ft, :],
                        start=(ft == 0), stop=False,
                    )
                nc.tensor.matmul(
                    psg[:, :], lhsT=ones_row_bf[0:1, :C], rhs=bg_sb[0:1, :],
                    start=False, stop=True,
                )
                eg = f32w.tile([C, H], F32, tag="eg", bufs=2)
                zg = f32w.tile([C, 1], F32, tag="zg", bufs=2)
                nc.scalar.activation(eg[:, :], psg[:, :], AF.Exp, accum_out=zg[:, :])
                rzg = f32w.tile([C, 1], F32, tag="rzg", bufs=2)
                nc.vector.reciprocal(rzg[:, :], zg[:, :])
                gatef = f32w.tile([C, H], F32, tag="gatef", bufs=2)
                nc.vector.tensor_scalar_mul(gatef[:, :], eg[:, :], rzg[:, :])
                psgt = psy.tile([H, C], F32, tag="psy", name="psgt")
                nc.tensor.transpose(psgt[:, :], gatef[:, :], ident[:C, :C])
                gft = f32w.tile([H, C], F32, tag="gft", bufs=2)
                nc.vector.tensor_copy(gft[:, :], psgt[:, :])
                grep = act.tile([H, MC], BF16, tag="grep", bufs=2)
                for sp in range(S):
                    nc.vector.tensor_copy(
                        grep[:, :].rearrange("h (b s) -> h b s", s=S)[:, :, sp],
                        gft[:, :],
                    )
                s["grep"] = grep
                # gate row gathered onto partition 0: [1, H*C] then
                # seq-replicated to [1, H*MC] (for the per-query 1/Z * gate row)
                growc = f32w.tile([1, H * C], F32, tag="growc", bufs=2)
                nc.sync.dma_start(growc[0:1, :], gft[:, :])
                growr = f32w.tile([1, H * MC], BF16, tag="growr", bufs=2)
                for sp in range(S):
                    nc.vector.tensor_copy(
                        growr[0:1, :].rearrange(
                            "o (h b s) -> o h b s", b=C, s=S)[:, :, :, sp],
                        growc[0:1, :].rearrange("o (h b) -> o h b", b=C),
                    )
                s["growr"] = growr


            # D: attention windows
            ysc = act.tile([P, TT, MC], BF16, tag="ysc", bufs=1)
            s["ysc"] = ysc
            wofs = []
            wo = 0
            for wn in WINS:
                wofs.append(wo)
                wo += wn

            def seg_dcorr():
                xt = s["xt"]
                for w in range(len(WINS)):
                    wn = WINS[w]
                    L = wn * S
                    woS = wofs[w] * S
                    psd = psr.tile([120, H], F32, tag="psr", name="psd")
                    for ft in range(FT):
                        nc.tensor.matmul(
                            psd[:L, :],
                            lhsT=xt[:, ft, woS:woS + L],
                            rhs=cq_sb[:, ft, :],
                            start=(ft == 0),
                            stop=(ft == FT - 1),
                        )
                    dsb = f32w.tile([120, H], F32, tag="dsb", bufs=4)
                    nc.scalar.copy(dsb[:L, :], psd[:L, :])
                    s[("dsb", w)] = dsb

            def seg_gate_dcorr():
                seg_gate()
                seg_dcorr()
            segs.append(seg_gate_dcorr)
            segs.extend(qm_segs[4:])

            def mk_attn_a(w):
                # scores + exp + mask for all heads
                def seg():
                    xt, qm_ = s["xt"], s["qm"]
                    wn = WINS[w]
                    L = wn * S
                    woS = wofs[w] * S
                    dsb = s[("dsb", w)]
                    abds = []
                    for h in range(H):
                        pss = psq.tile([P, 512], F32, tag="psq")
                        for dt in range(FT):
                            nc.tensor.matmul(
                                pss[:L, :L],
                                lhsT=xt[:, dt, woS:woS + L],
                                rhs=qm_[:, h * FT + dt, woS:woS + L],
                                start=(dt == 0),
                                stop=(dt == FT - 1),
                            )
                        es = act.tile([120, 128], BF16, tag="es", bufs=3)
                        nc.scalar.activation(
                            es[:L, :L], pss[:L, :L], AF.Exp,
                            bias=dsb[:L, h:h + 1],
                        )
                        abd = act.tile([120, 128], BF16, tag="abd", bufs=5)
                        nc.vector.tensor_mul(abd[:L, :L], es[:L, :L], mask_bd[:L, :L])
                        abds.append(abd)
                    s[("abds", w)] = abds
                return seg

            def mk_attn_b(w):
                # Z colsums, 1/Z, gate row, broadcast, scaled attn weights
                def seg():
                    wn = WINS[w]
                    L = wn * S
                    woS = wofs[w] * S
                    abds = s[("abds", w)]
                    psz = psr.tile([1, 512], F32, tag="psr")
                    for h in range(H):
                        nc.tensor.matmul(
                            psz[0:1, h * L:h * L + L],
                            lhsT=ones_col_bf[:L, 0:1],
                            rhs=abds[h][:L, :L],
                            start=True, stop=True,
                        )
                    rz = f32w.tile([1, 512], F32, tag="rz", bufs=2)
                    nc.vector.reciprocal(rz[0:1, :H * L], psz[0:1, :H * L])
                    wrow = f32w.tile([1, 512], F32, tag="wrow", bufs=2)
                    nc.vector.tensor_mul(
                        wrow[0:1, :H * L].rearrange("o (h m) -> o h m", m=L),
                        rz[0:1, :H * L].rearrange("o (h m) -> o h m", m=L),
                        s["growr"][0:1, :].rearrange(
                            "o (h m) -> o h m", m=MC)[:, :, woS:woS + L],
                    )
                    wbc = f32w.tile([P, 512], F32, tag="wbc", bufs=2)
                    for h in range(H):
                        nc.gpsimd.partition_broadcast(
                            wbc[:, h * L:h * L + L], wrow[0:1, h * L:h * L + L]
                        )
                    abscs = []
                    for h in range(H):
                        absc = act.tile([120, 128], BF16, tag="absc", bufs=6)
                        nc.vector.tensor_mul(
                            absc[:L, :L], abds[h][:L, :L], wbc[:L, h * L:h * L + L]
                        )
                        abscs.append(absc)
                    s[("abscs", w)] = abscs
                return seg

            def mk_attn_c(w, fts):
                # attnx for feature tiles in fts + evac into ysc
                def seg():
                    xw = s["xw"]
                    wn = WINS[w]
                    L = wn * S
                    woS = wofs[w] * S
                    abscs = s[("abscs", w)]
                    for ft in fts:
                        pyt = psy.tile([P, 512], F32, tag="psy")
                        for h in range(H):
                            nc.tensor.matmul(
                                pyt[:, h * L:h * L + L],
                                lhsT=xw[:L, w, ft * P:(ft + 1) * P],
                                rhs=abscs[h][:L, :L],
                                start=True, stop=True,
                            )
                        dst = ysc[:, :, woS:woS + L].rearrange(
                            "p (h f) m -> p h f m", f=FT)[:, :, ft, :]
                        src = pyt[:, :H * L].rearrange("p (h m) -> p h m", m=L)
                        if ft % 2 == 0:
                            nc.vector.tensor_copy(dst, src)
                        else:
                            nc.scalar.copy(dst, src)
                return seg

            for w in range(len(WINS)):
                segs.append(mk_attn_a(w))
                segs.append(mk_attn_b(w))
                segs.append(mk_attn_c(w, (0, 1)))
                segs.append(mk_attn_c(w, (2, 3)))

            # E: fused projection + residual (LN1 stats deferred one dp so
            # the PE never waits on the stt/sq of the dp it just produced)
            x1u = f32w.tile([P, DPT, MC], F32R, tag="x1u", bufs=1)
            sq1 = f32w.tile([P, DPT, MC], F32R, tag="sq1", bufs=1)
            s["x1u"], s["sq1"] = x1u, sq1

            def stats1(dp):
                nc.tensor.matmul(
                    s["pstat1"][0:1, :MC],
                    lhsT=ones_col_f32[:, 0:1],
                    rhs=x1u[:, dp, :],
                    start=(dp == 0), stop=(dp == DPT - 1),
                )
                nc.tensor.matmul(
                    s["psqs1"][0:1, :MC],
                    lhsT=ones_col_f32[:, 0:1],
                    rhs=sq1[:, dp, :],
                    start=(dp == 0), stop=(dp == DPT - 1),
                )

            def mk_proj(dp):
                def seg():
                    if dp == 0:
                        # allocated here (not at construction) so the psr
                        # rotation order matches emission order
                        s["pstat1"] = psr.tile([1, 512], F32, tag="psr", name="pstat1")
                        s["psqs1"] = psy.tile([1, 512], F32, tag="psy", name="psqs1")
                    xt = s["xt"]
                    ps = psb.tile([P, 512], F32, tag="psb")
                    for t in range(TT):
                        nc.tensor.matmul(
                            ps[:, :MC],
                            lhsT=wvp_sb[:, t, dp * P:(dp + 1) * P],
                            rhs=ysc[:, t, :],
                            start=(t == 0), stop=False,
                        )
                    nc.tensor.matmul(
                        ps[:, :MC],
                        lhsT=cg_sb[:, dp * P:(dp + 1) * P],
                        rhs=s["grep"][:, :],
                        start=False, stop=True,
                    )
                    nc.vector.scalar_tensor_tensor(
                        out=x1u[:, dp, :],
                        in0=ps[:, :MC],
                        scalar=bpc_sb[:, dp:dp + 1],
                        in1=xt[:, dp, :],
                        op0=ALU.add,
                        op1=ALU.add,
                    )
                    nc.scalar.activation(sq1[:, dp, :], x1u[:, dp, :], AF.Square)
                    if dp > 0:
                        stats1(dp - 1)
                return seg
            for dp in range(DPT):
                segs.append(mk_proj(dp))
            return segs

        # LN1 of chunk ch, emitted early in iteration ch+1 (3-stage pipeline)
        def mid_segments(ch):
            s = st[ch]
            x1n = act.tile([P, DPT, MC], BF16, tag="x1n", bufs=1)
            s["x1n"] = x1n

            def seg_e_tail():
                dp = DPT - 1
                nc.tensor.matmul(
                    s["pstat1"][0:1, :MC],
                    lhsT=ones_col_f32[:, 0:1],
                    rhs=s["x1u"][:, dp, :],
                    start=(dp == 0), stop=(dp == DPT - 1),
                )
                nc.tensor.matmul(
                    s["psqs1"][0:1, :MC],
                    lhsT=ones_col_f32[:, 0:1],
                    rhs=s["sq1"][:, dp, :],
                    start=(dp == 0), stop=(dp == DPT - 1),
                )

            def seg_ln1_chain():
                rs_bc, nm_bc = ln_chain(s["pstat1"], s["psqs1"], tag="1")
                s["rs1"], s["nm1"] = rs_bc, nm_bc

            def seg_ln1_apply():
                ln_apply(s["x1u"], s["rs1"], s["nm1"],
                         lambda dp: x1n[:, dp, :], 0)
            return [seg_e_tail, seg_ln1_chain, seg_ln1_apply]

        # shared LN helpers (T-layout; stats already in pstat rows {0, 32}).
        # rstd = rsqrt(var) via the quake bit trick + 2 Newton steps, all on
        # GPSIMD so the Act engine never leaves the exp table set (eps is
        # dropped: row variance here is O(1), so eps=1e-5 is far below the
        # bf16 noise floor).
        def ln_chain(pstat, psqs, tag):
            U32 = mybir.dt.uint32
            # negmean = -sum/D (sign is irrelevant for the square)
            mean = f32w.tile([1, 512], F32, tag="mean", bufs=1)
            nc.vector.tensor_scalar_mul(mean[0:1, :MC], pstat[0:1, :MC], -1.0 / D)
            msq = f32w.tile([1, 512], F32, tag="msq", bufs=1)
            nc.scalar.activation(msq[0:1, :MC], mean[0:1, :MC], AF.Square)
            var = f32w.tile([1, 512], F32, tag="var", bufs=1)
            nc.vector.scalar_tensor_tensor(
                out=var[0:1, :MC], in0=psqs[0:1, :MC], scalar=1.0 / D,
                in1=msq[0:1, :MC], op0=ALU.mult, op1=ALU.subtract,
            )
            std = f32w.tile([1, 512], F32, tag="std", bufs=1)
            nc.scalar.activation(std[0:1, :MC], var[0:1, :MC], AF.Sqrt,
                                 bias=eps_sb[0:1, 0:1])
            rs_row = f32w.tile([1, 512], F32, tag="rsr", bufs=1)
            nc.vector.reciprocal(rs_row[0:1, :MC], std[0:1, :MC])
            # nm = -mean*rstd = negmean*rstd
            nm_row = f32w.tile([1, 512], F32, tag="nmr", bufs=1)
            nc.gpsimd.tensor_mul(nm_row[0:1, :MC], mean[0:1, :MC], rs_row[0:1, :MC])
            rs_bc = f32w.tile([P, MC], F32, tag="rsb", bufs=1)
            nc.gpsimd.partition_broadcast(rs_bc[:, :], rs_row[0:1, :MC])
            nm_bc = f32w.tile([P, MC], F32, tag="nmb", bufs=1)
            nc.gpsimd.partition_broadcast(nm_bc[:, :], nm_row[0:1, :MC])
            return rs_bc, nm_bc

        def ln_apply(xu, rs_bc, nm_bc, dst, ln_row):
            for dp in range(DPT):
                tmp = f32w.tile([P, MC], F32R, tag="lntmp", bufs=2)
                nc.vector.tensor_mul(tmp[:, :], xu[:, dp, :], rs_bc[:, :])
                if apply_ln_affine:
                    t2 = f32w.tile([P, MC], F32R, tag="lnt2", bufs=2)
                    nc.gpsimd.tensor_add(t2[:, :], tmp[:, :], nm_bc[:, :])
                    nc.vector.tensor_scalar(
                        out=dst(dp), in0=t2[:, :],
                        scalar1=ln_sb[:, ln_row, dp:dp + 1],
                        scalar2=ln_sb[:, ln_row + 1, dp:dp + 1],
                        op0=ALU.mult, op1=ALU.add,
                    )
                else:
                    nc.gpsimd.tensor_add(dst(dp), tmp[:, :], nm_bc[:, :])

        # ---------------- back half: FFN, LN2, store -------------------------
        def back_segments(ch):
            s = st[ch]
            m0 = ch * MC
            segs = []
            pso = [psb.tile([P, 512], F32, tag="psb", name=f"pso{ch}_{i}")
                   for i in range(DPT)]
            s["pso"] = pso

            def ffn1_half(d1):
                x1n = s["x1n"]
                psf = psq.tile([P, 512], F32, tag="psq")
                for ft in range(FT):
                    nc.tensor.matmul(
                        psf[:, :MC],
                        lhsT=w1_sb[:, ft, d1 * P:(d1 + 1) * P],
                        rhs=x1n[:, ft, :],
                        start=(ft == 0),
                        stop=(ft == FT - 1),
                    )
                hrelu = act.tile([P, MC], BF16, tag="hrelu", bufs=6)
                if d1 % 2 == 0:
                    nc.vector.tensor_scalar(
                        out=hrelu[:, :], in0=psf[:, :MC],
                        scalar1=b1c_sb[:, d1:d1 + 1], scalar2=0.0,
                        op0=ALU.add, op1=ALU.max,
                    )
                else:
                    nc.scalar.activation(
                        hrelu[:, :], psf[:, :MC], AF.Relu,
                        bias=b1c_sb[:, d1:d1 + 1],
                    )
                return hrelu

            def ffn2_half(d1, hrelu):
                for dp in range(DPT):
                    nc.tensor.matmul(
                        pso[dp][:, :MC],
                        lhsT=w2_sb[:, d1, dp * P:(dp + 1) * P],
                        rhs=hrelu[:, :],
                        start=(d1 == 0),
                        stop=(d1 == D1T - 1),
                    )

            def mk_ffn_pair(d1):
                def seg():
                    ha = ffn1_half(d1)
                    hb = ffn1_half(d1 + 1)
                    ffn2_half(d1, ha)
                    ffn2_half(d1 + 1, hb)
                return seg
            for d1 in range(0, D1T, 2):
                segs.append(mk_ffn_pair(d1))

            # H1: residual + LN2 stats
            x2u = f32w.tile([P, DPT, MC], F32R, tag="x2u", bufs=1)
            sq2 = f32w.tile([P, DPT, MC], F32R, tag="sq2", bufs=1)
            x2n = f32w.tile([P, DPT, MC], F32, tag="x2n", bufs=1)

            def seg_h1a():
                x1n = s["x1n"]
                for dp in range(DPT):
                    nc.vector.scalar_tensor_tensor(
                        out=x2u[:, dp, :],
                        in0=pso[dp][:, :MC],
                        scalar=b2c_sb[:, dp:dp + 1],
                        in1=x1n[:, dp, :],
                        op0=ALU.add,
                        op1=ALU.add,
                    )
                    nc.scalar.activation(sq2[:, dp, :], x2u[:, dp, :], AF.Square)
            segs.append(seg_h1a)

            def seg_h1b():
                pstat2 = psr.tile([1, 512], F32, tag="psr", name="pstat2")
                psqs2 = psy.tile([1, 512], F32, tag="psy", name="psqs2")
                s["pstat2"], s["psqs2"] = pstat2, psqs2
                for dp in range(DPT):
                    nc.tensor.matmul(
                        pstat2[0:1, :MC],
                        lhsT=ones_col_f32[:, 0:1],
                        rhs=x2u[:, dp, :],
                        start=(dp == 0), stop=(dp == DPT - 1),
                    )
                    nc.tensor.matmul(
                        psqs2[0:1, :MC],
                        lhsT=ones_col_f32[:, 0:1],
                        rhs=sq2[:, dp, :],
                        start=(dp == 0), stop=(dp == DPT - 1),
                    )
            segs.append(seg_h1b)

            def seg_h2():
                rs_bc, nm_bc = ln_chain(s["pstat2"], s["psqs2"], tag="2")
                s["rs2"], s["nm2"] = rs_bc, nm_bc
            segs.append(seg_h2)

            def seg_h3():
                ln_apply(x2u, s["rs2"], s["nm2"],
                         lambda dp: x2n[:, dp, :], 2)
            segs.append(seg_h3)

            def seg_store():
                for dp in range(DPT):
                    nc.sync.dma_start(
                        out_flat[dp * P:(dp + 1) * P, m0:m0 + MC],
                        x2n[:, dp, :],
                    )
                st.pop(ch, None)
            segs.append(seg_store)
            return segs

        # ---------------- emission: 3-stage software pipeline ----------------
        # front seg indices: B=0..15, C=16, D=17..28 (4 per window), E=29..32
        # mid (ch-1): 0=E-tail stats, 1=LN1 chain, 2=LN1 apply
        # back (ch-1): G=0..15 (ffn d1), H1=16, H2=17, H3=18, I=19
        insert_after = {
            2: [("m", 0)],                              # LN1 dp3 stats
            3: [("m", 1)],                              # LN1 chain
            11: [("m", 2)],                             # LN1 apply -> x1n
            16: [("b", 0)], 17: [("b", 1)], 18: [("b", 2)], 19: [("b", 3)],
            20: [("b", 4)], 21: [("b", 5)], 22: [("b", 6)], 23: [("b", 7)],
            24: [("b", 8)],                             # H1a: x2u evac (DVE)
            25: [("b", 9)],                             # H1b: LN2 stats
            26: [("b", 10)],                            # H2: LN2 chain
            28: [("b", 11)],                            # H3: LN2 apply
            29: [("b", 12)],                            # store
        }

        load_xt(0)
        load_big_weights(0)      # m
        load_xw(0)
        load_big_weights(1)      # wvp
        if nch > 1:
            load_xt(1)
            load_xw(1)
        load_big_weights(2)      # w1, w2
        prev_back = None
        prev_ch = None
        for ch in range(nch):
            fsegs = front_segments(ch)
            mids = mid_segments(prev_ch) if prev_ch is not None else None
            if pipeline and prev_back is not None:
                done = set()
                for fi, fseg in enumerate(fsegs):
                    fseg()
                    for kind, bi in insert_after.get(fi, ()):
                        (mids if kind == "m" else prev_back)[bi]()
                        done.add((kind, bi))
                for bi in range(len(mids)):
                    if ("m", bi) not in done:
                        mids[bi]()
                for bi in range(len(prev_back)):
                    if ("b", bi) not in done:
                        prev_back[bi]()
            else:
                if mids is not None:
                    for mseg in mids:
                        mseg()
                if prev_back is not None:
                    for bseg in prev_back:
                        bseg()
                for fseg in fsegs:
                    fseg()
            if ch + 2 < nch:
                load_xt(ch + 2)
                load_xw(ch + 2)
            prev_back = back_segments(ch)
            prev_ch = ch
        for mseg in mid_segments(prev_ch):
            mseg()
        for bseg in prev_back:
            bseg()

        _stack.close()

    nc.compile()
    return nc


def _prep_inputs(inputs):
    """Host-side weight fusion; returns per-core in_maps."""
    bf = ml_dtypes.bfloat16
    x = np.ascontiguousarray(inputs["x"], dtype=np.float32)
    Wq = inputs["Wq"].astype(np.float32)
    Wk = inputs["Wk"].astype(np.float32)
    Wv = inputs["Wv"].astype(np.float32)
    Wp = inputs["Wp"].astype(np.float32).reshape(H, D, D)
    sc = 1.0 / math.sqrt(D)
    # M_h = Wq_h Wk_h^T / sqrt(D), stacked head-major on columns: [F, H*F]
    M = np.einsum("hfd,hgd->hfg", Wq, Wk) * sc
    m_p = np.ascontiguousarray(M.transpose(1, 0, 2).reshape(F, H * F)).astype(bf)
    # c_h = Wk_h bq_h / sqrt(D): per-key additive bias -> [F, H]
    cq_p = (np.einsum("hfd,hd->hf", Wk, inputs["bq"].astype(np.float32))
            * sc).T.astype(bf)
    cq_p = np.ascontiguousarray(cq_p)
    wvp_p = np.einsum("hfd,hde->hfe", Wv, Wp).reshape(H * F, D).astype(bf)
    cg_p = np.einsum("hd,hde->he", inputs["bv"].astype(np.float32), Wp).astype(bf)
    w1_p = inputs["W1"].astype(bf)
    w2_p = inputs["W2"].astype(bf)
    wg_p = (inputs["Wg"].astype(np.float32) / S).astype(bf)

    def col(v, nt):
        return np.ascontiguousarray(v.astype(np.float32).reshape(nt, 128).T)

    bpc_p = col(inputs["bp"], DPT)
    b1c_p = col(inputs["b1"], D1T)
    b2c_p = col(inputs["b2"], DPT)
    bg_p = inputs["bg"].astype(np.float32).reshape(1, H).astype(bf)
    ln_p = np.stack(
        [inputs["g1"], inputs["be1"], inputs["g2"], inputs["be2"]]
    ).astype(np.float32)
    apply_affine = not (
        np.all(ln_p[0] == 1) and np.all(ln_p[1] == 0)
        and np.all(ln_p[2] == 1) and np.all(ln_p[3] == 0)
    )
    shared = dict(
        m_p=m_p, cq_p=cq_p, wvp_p=wvp_p, cg_p=cg_p, w1_p=w1_p, w2_p=w2_p,
        wg_p=wg_p, bpc_p=bpc_p, b1c_p=b1c_p, b2c_p=b2c_p, bg_p=bg_p,
        mask_p=_make_mask(),
    )
    if apply_affine:
        shared["ln_p"] = ln_p
    x_bf = x.reshape(-1, F).astype(bf)
    in_maps = []
    for c in range(NCORES):
        m = dict(shared)
        m["x_bf"] = np.ascontiguousarray(x_bf[c * BC * S:(c + 1) * BC * S])
        in_maps.append(m)
    return in_maps, apply_affine


def _prep_inputs_small(inputs, nsamp):
    """Single map covering the first nsamp samples (for CoreSim tests)."""
    sub = dict(inputs)
    sub["x"] = np.asarray(inputs["x"])[:nsamp]
    maps, apply_affine = _prep_inputs(sub)
    m = maps[0]
    m["x_bf"] = m["x_bf"][: nsamp * S]
    return m, apply_affine


def _make_mask():
    m = np.zeros((120, 120), dtype=np.float32)
    for b in range(12):
        m[10 * b:10 * b + 10, 10 * b:10 * b + 10] = 1.0
    return m.astype(ml_dtypes.bfloat16)


_CACHED = {}


def _get_kernel(apply_affine):
    key = apply_affine
    if key not in _CACHED:
        _CACHED[key] = build_kernel(apply_affine)
    return _CACHED[key]


def kernel(**inputs):
    from concourse.bass_utils import run_bass_kernel_spmd

    in_maps, apply_affine = _prep_inputs(inputs)
    nc = _get_kernel(apply_affine)
    res = run_bass_kernel_spmd(nc, in_maps, list(range(NCORES)))
    outs = [
        np.asarray(r["out"]).reshape(D, BC * S).T.reshape(BC, S, F)
        for r in res.results
    ]
    return np.concatenate(outs, axis=0)


if __name__ == "__main__":
    nc = build_kernel(False)
    print("built ok")


# revision 5
# speedup vs baseline: 1.4412x; 1.0000x over previous
"""Trainium2 Bass kernel for nn_AttnBlock (dense transformer block), v2.

Strategy (pure data-parallel over batch, 8 cores):
  - Each core gets B/8 = 512 samples; all weights replicated.
  - Algebraic fusion (host-side, weights only):
      * K projection eliminated: scores = x (Wq Wk^T/sqrt(D)) x^T per head.
        Qm = x @ M with M_h = Wq_h Wk_h^T / sqrt(D) replaces BOTH Q and K.
        The bq-side bias term (Wk_h bq_h)·x_key is applied as a per-key
        additive bias on the exp (softmax-row-shift removes the bk terms).
      * attention applied to raw x:  y_h = attn_h @ x    (per sample)
      * V-projection and output projection fused: Wvp_h = Wv_h @ Wp_h
      * per-head bias folded via the gate vector: proj += gate @ (bv_h Wp_h)
      * gate/softmax normalization folded into the attention weights
        (pre-attnx), so PSUM evacuations are plain copies (DVE/Act split).
  - GPSIMD (Pool) used for partition-broadcasts and LN applies.
  - Output stored in T-layout [D, rows]; transposed on host.
  - 2-stage software pipeline: chunk n's FFN is interleaved into chunk
    n+1's front half to keep the PE busy across dependency gaps.

Self-contained: hardcodes shapes; imports only the concourse stack.
"""

import math
import os
import sys

import numpy as np

for _p in ("/opt/trn_rl_repo", os.path.expanduser("~/.axon_site/_ro/trn_rl_repo")):
    if os.path.isdir(_p) and _p not in sys.path:
        sys.path.insert(0, _p)

import ml_dtypes  # noqa: E402

import concourse.bass as bass  # noqa: E402
import concourse.mybir as mybir  # noqa: E402
import concourse.tile as tile  # noqa: E402
from concourse import bacc  # noqa: E402
from concourse.masks import make_identity  # noqa: E402

F32 = mybir.dt.float32
BF16 = mybir.dt.bfloat16
F32R = mybir.dt.float32r
AF = mybir.ActivationFunctionType
ALU = mybir.AluOpType

# Problem shapes (hardcoded per spec)
B, S, F, D, H = 4096, 10, 512, 512, 4
EPS = 1e-5
NCORES = 8
BC = B // NCORES          # samples per core = 512
P = 128

# Tiling
C = 32                    # samples per chunk
NCH = BC // C             # 16 chunks
MC = C * S                # 320 rows per chunk
WINS = (12, 12, 8)        # samples per attention window (sum = C)
FT = F // P               # 4 input-feature tiles
TT = (H * F) // P         # 16 Qm tiles (head-major over x-features)
D1T = (4 * D) // P        # 16 ffn hidden tiles
DPT = D // P              # 4 d_model tiles


def build_kernel(apply_ln_affine: bool, nch: int = NCH, debug: bool = False,
                 pipeline: bool = True):
    MR = nch * MC  # rows handled by this program
    nc = bacc.Bacc(None, target_bir_lowering=False, debug=debug)
    names = {}

    _lp = nc.allow_low_precision(reason="float32r intermediates are 4-byte")
    _lp.__enter__()
    with tile.TileContext(nc) as tc:
        with tc.tile_pool(name="dram", bufs=1, space="DRAM") as dram:
            x_bf = dram.tile([MR, F], BF16, kind="ExternalInput", name="x_bf", uniquify=False)
            m_d = dram.tile([F, H * F], BF16, kind="ExternalInput", name="m_p", uniquify=False)
            wvp_d = dram.tile([H * F, D], BF16, kind="ExternalInput", name="wvp_p", uniquify=False)
            w1_d = dram.tile([D, 4 * D], BF16, kind="ExternalInput", name="w1_p", uniquify=False)
            w2_d = dram.tile([4 * D, D], BF16, kind="ExternalInput", name="w2_p", uniquify=False)
            wg_d = dram.tile([F, H], BF16, kind="ExternalInput", name="wg_p", uniquify=False)
            cg_d = dram.tile([H, D], BF16, kind="ExternalInput", name="cg_p", uniquify=False)
            cq_d = dram.tile([F, H], BF16, kind="ExternalInput", name="cq_p", uniquify=False)
            bpc_d = dram.tile([P, DPT], F32, kind="ExternalInput", name="bpc_p", uniquify=False)
            b1c_d = dram.tile([P, D1T], F32, kind="ExternalInput", name="b1c_p", uniquify=False)
            b2c_d = dram.tile([P, DPT], F32, kind="ExternalInput", name="b2c_p", uniquify=False)
            bg_d = dram.tile([1, H], BF16, kind="ExternalInput", name="bg_p", uniquify=False)
            mask_d = dram.tile([120, 120], BF16, kind="ExternalInput", name="mask_p", uniquify=False)
            if apply_ln_affine:
                ln_d = dram.tile([4, D], F32, kind="ExternalInput", name="ln_p", uniquify=False)
            # output in T-layout: [D, rows]; transposed on host
            out_d = dram.tile([D, MR], F32, kind="ExternalOutput", name="out", uniquify=False)
        names["out"] = "out"

        from contextlib import ExitStack
        _stack = ExitStack()
        const = _stack.enter_context(tc.tile_pool(name="const", bufs=1))
        wts = _stack.enter_context(tc.tile_pool(name="wts", bufs=1))
        act = _stack.enter_context(tc.tile_pool(name="act", bufs=1))
        f32w = _stack.enter_context(tc.tile_pool(name="f32w", bufs=1))
        psq = _stack.enter_context(tc.tile_pool(name="psq", bufs=2, space="PSUM"))
        psb = _stack.enter_context(tc.tile_pool(name="psb", bufs=4, space="PSUM"))
        psr = _stack.enter_context(tc.tile_pool(name="psr", bufs=1, space="PSUM"))
        psy = _stack.enter_context(tc.tile_pool(name="psy", bufs=1, space="PSUM"))

        # ---- constants ----
        ident = const.tile([P, P], F32, tag="ident")
        make_identity(nc, ident)
        ones_row_bf = const.tile([1, 512], BF16, tag="ones_row_bf")
        nc.vector.memset(ones_row_bf[:], 1.0)
        ones_tmp = const.tile([P, P], F32, tag="ones_tmp")
        nc.vector.memset(ones_tmp[:], 1.0)
        ones_col_f32 = const.tile([P, 1], F32R, tag="ones_col_f32")
        nc.vector.tensor_copy(ones_col_f32[:], ones_tmp[:, 0:1])
        ones_col_bf = const.tile([P, 1], BF16, tag="ones_col_bf")
        nc.vector.memset(ones_col_bf[:], 1.0)
        eps_sb = const.tile([1, 1], F32, tag="eps")
        nc.vector.memset(eps_sb[:], EPS)
        # f32 whose bits are the rsqrt seed magic 0x5f3759df
        _magicf = float(np.frombuffer(
            np.uint32(0x5F3759DF).tobytes(), np.float32)[0])
        magic_sb = const.tile([1, 512], F32, tag="magic")
        nc.vector.memset(magic_sb[:], _magicf)
        neghalf_sb = const.tile([1, 512], F32, tag="neghalf")
        nc.vector.memset(neghalf_sb[:], -0.5)
        c15_sb = const.tile([1, 512], F32, tag="c15")
        nc.vector.memset(c15_sb[:], 1.5)
        mask_bd = const.tile([120, 120], BF16, tag="mask_bd")
        nc.sync.dma_start(mask_bd[:], mask_d[:])

        # ---- resident weights ----
        m_sb = wts.tile([P, FT, H * F], BF16, tag="m")
        wvp_sb = wts.tile([P, TT, D], BF16, tag="wvp")
        w1_sb = wts.tile([P, FT, 4 * D], BF16, tag="w1")
        w2_sb = wts.tile([P, D1T, D], BF16, tag="w2")
        wg_sb = wts.tile([P, FT, H], BF16, tag="wg")
        cg_sb = wts.tile([H, D], BF16, tag="cg")
        cq_sb = wts.tile([P, FT, H], BF16, tag="cq")
        bpc_sb = wts.tile([P, DPT], F32, tag="bpc")
        b1c_sb = wts.tile([P, D1T], F32, tag="b1c")
        b2c_sb = wts.tile([P, DPT], F32, tag="b2c")
        bg_sb = wts.tile([1, H], BF16, tag="bg")
        # small tensors first so early chunks aren't blocked behind big DMAs
        nc.sync.dma_start(bpc_sb[:], bpc_d[:])
        nc.sync.dma_start(b1c_sb[:], b1c_d[:])
        nc.sync.dma_start(b2c_sb[:], b2c_d[:])
        nc.sync.dma_start(bg_sb[:], bg_d[:])
        nc.sync.dma_start(cq_sb[:], cq_d[:].rearrange("(t p) n -> p t n", p=P))
        nc.sync.dma_start(wg_sb[:], wg_d[:].rearrange("(t p) n -> p t n", p=P))
        nc.sync.dma_start(cg_sb[:], cg_d[:])
        if apply_ln_affine:
            ln_sb = wts.tile([P, 4, DPT], F32, tag="ln")
            nc.sync.dma_start(ln_sb[:], ln_d[:].rearrange("r (t p) -> p r t", p=P))

        def load_big_weights(stage):
            # staged + quartered so chunk 0's compute starts after only the
            # first quarter of m (+xt) arrives
            if stage == 0:
                for q in range(4):
                    nc.sync.dma_start(
                        m_sb[:, :, q * 512:(q + 1) * 512],
                        m_d[:, q * 512:(q + 1) * 512].rearrange(
                            "(t p) n -> p t n", p=P))
            elif stage == 1:
                nc.sync.dma_start(
                    wvp_sb[:], wvp_d[:].rearrange("(t p) n -> p t n", p=P))
            elif stage == 2:
                for q in range(4):
                    nc.sync.dma_start(
                        w1_sb[:, :, q * 512:(q + 1) * 512],
                        w1_d[:, q * 512:(q + 1) * 512].rearrange(
                            "(t p) n -> p t n", p=P))
                for q in range(4):
                    nc.sync.dma_start(
                        w2_sb[:, q * 4:(q + 1) * 4, :],
                        w2_d[q * 512:(q + 1) * 512, :].rearrange(
                            "(t p) n -> p t n", p=P))

        x_flat = x_bf[:]
        out_flat = out_d[:]

        # per-chunk state passed between segments (keyed by chunk index)
        st = {}

        def load_xt(ch):
            m0 = ch * MC
            xt = act.tile([P, FT, MC], BF16, tag="xt", bufs=3)
            for ft in range(FT):
                nc.sync.dma_start(
                    xt[:, ft, :],
                    x_flat[m0:m0 + MC, ft * P:(ft + 1) * P],
                    transpose=True,
                )
            st.setdefault(ch, {})["xt"] = xt

        def load_xw(ch):
            m0 = ch * MC
            xw = act.tile([120, len(WINS), F], BF16, tag="xw", bufs=3)
            wo = 0
            for w, wn in enumerate(WINS):
                nc.sync.dma_start(
                    xw[:wn * S, w, :], x_flat[m0 + wo * S:m0 + (wo + wn) * S, :]
                )
                wo += wn
            st.setdefault(ch, {})["xw"] = xw

        # ---------------- front half: Qm, gate, attention, Wvp, LN1 ----------
        def front_segments(ch):
            s = st[ch]
            segs = []

            # B: Qm projection, one segment per output tile
            qm = act.tile([P, TT, MC], BF16, tag="qm", bufs=1)
            s["qm"] = qm

            def mk_qm(t):
                def seg():
                    xt = s["xt"]
                    ps = psq.tile([P, 512], F32, tag="psq")
                    for ft in range(FT):
                        nc.tensor.matmul(
                            ps[:, :MC],
                            lhsT=m_sb[:, ft, t * P:(t + 1) * P],
                            rhs=xt[:, ft, :],
                            start=(ft == 0),
                            stop=(ft == FT - 1),
                        )
                    if t % 2 == 0:
                        nc.vector.tensor_copy(qm[:, t, :], ps[:, :MC])
                    else:
                        nc.scalar.copy(qm[:, t, :], ps[:, :MC])
                return seg
            qm_segs = [mk_qm(t) for t in range(TT)]
            segs.extend(qm_segs[:4])

            # C: gate = softmax(mean(x) @ Wg + bg), replicated over s
            def seg_gate():
                xt = s["xt"]
                xm_bf = act.tile([P, FT, C], BF16, tag="xm", bufs=2)
                for ft in range(FT):
                    nc.vector.tensor_reduce(
                        xm_bf[:, ft, :],
                        xt[:, ft, :].rearrange("p (b s) -> p b s", s=S),
                        axis=mybir.AxisListType.X,
                        op=ALU.add,
                    )
                psg = psy.tile([C, H], F32, tag="psy", name="psg")
                for ft in range(FT):
                    nc.tensor.matmul(
                        psg[:, :], lhsT=xm_bf[:, ft, :], rhs=wg_sb[:, ft, :],
                        start=(ft == 0), stop=False,
                    )
                nc.tensor.matmul(
                    psg[:, :], lhsT=ones_row_bf[0:1, :C], rhs=bg_sb[0:1, :],
                    start=False, stop=True,
                )
                eg = f32w.tile([C, H], F32, tag="eg", bufs=2)
                zg = f32w.tile([C, 1], F32, tag="zg", bufs=2)
                nc.scalar.activation(eg[:, :], psg[:, :], AF.Exp, accum_out=zg[:, :])
                rzg = f32w.tile([C, 1], F32, tag="rzg", bufs=2)
                nc.vector.reciprocal(rzg[:, :], zg[:, :])
                gatef = f32w.tile([C, H], F32, tag="gatef", bufs=2)
                nc.vector.tensor_scalar_mul(gatef[:, :], eg[:, :], rzg[:, :])
                psgt = psy.tile([H, C], F32, tag="psy", name="psgt")
                nc.tensor.transpose(psgt[:, :], gatef[:, :], ident[:C, :C])
                gft = f32w.tile([H, C], F32, tag="gft", bufs=2)
                nc.vector.tensor_copy(gft[:, :], psgt[:, :])
                grep = act.tile([H, MC], BF16, tag="grep", bufs=2)
                for sp in range(S):
                    nc.vector.tensor_copy(
                        grep[:, :].rearrange("h (b s) -> h b s", s=S)[:, :, sp],
                        gft[:, :],
                    )
                s["grep"] = grep
                # gate row gathered onto partition 0: [1, H*C] then
                # seq-replicated to [1, H*MC] (for the per-query 1/Z * gate row)
                growc = f32w.tile([1, H * C], F32, tag="growc", bufs=2)
                nc.sync.dma_start(growc[0:1, :], gft[:, :])
                growr = f32w.tile([1, H * MC], BF16, tag="growr", bufs=2)
                for sp in range(S):
                    nc.vector.tensor_copy(
                        growr[0:1, :].rearrange(
                            "o (h b s) -> o h b s", b=C, s=S)[:, :, :, sp],
                        growc[0:1, :].rearrange("o (h b) -> o h b", b=C),
                    )
                s["growr"] = growr


            # D: attention windows
            ysc = act.tile([P, TT, MC], BF16, tag="ysc", bufs=1)
            s["ysc"] = ysc
            wofs = []
            wo = 0
            for wn in WINS:
                wofs.append(wo)
                wo += wn

            def seg_dcorr():
                xt = s["xt"]
                for w in range(len(WINS)):
                    wn = WINS[w]
                    L = wn * S
                    woS = wofs[w] * S
                    psd = psr.tile([120, H], F32, tag="psr", name="psd")
                    for ft in range(FT):
                        nc.tensor.matmul(
                            psd[:L, :],
                            lhsT=xt[:, ft, woS:woS + L],
                            rhs=cq_sb[:, ft, :],
                            start=(ft == 0),
                            stop=(ft == FT - 1),
                        )
                    dsb = f32w.tile([120, H], F32, tag="dsb", bufs=4)
                    nc.scalar.copy(dsb[:L, :], psd[:L, :])
                    s[("dsb", w)] = dsb

            def seg_gate_dcorr():
                seg_gate()
                seg_dcorr()
            segs.append(seg_gate_dcorr)
            segs.extend(qm_segs[4:])

            def mk_attn_a(w):
                # scores + exp + mask for all heads
                def seg():
                    xt, qm_ = s["xt"], s["qm"]
                    wn = WINS[w]
                    L = wn * S
                    woS = wofs[w] * S
                    dsb = s[("dsb", w)]
                    abds = []
                    for h in range(H):
                        pss = psq.tile([P, 512], F32, tag="psq")
                        for dt in range(FT):
                            nc.tensor.matmul(
                                pss[:L, :L],
                                lhsT=xt[:, dt, woS:woS + L],
                                rhs=qm_[:, h * FT + dt, woS:woS + L],
                                start=(dt == 0),
                                stop=(dt == FT - 1),
                            )
                        es = act.tile([120, 128], BF16, tag="es", bufs=3)
                        nc.scalar.activation(
                            es[:L, :L], pss[:L, :L], AF.Exp,
                            bias=dsb[:L, h:h + 1],
                        )
                        abd = act.tile([120, 128], BF16, tag="abd", bufs=5)
                        nc.vector.tensor_mul(abd[:L, :L], es[:L, :L], mask_bd[:L, :L])
                        abds.append(abd)
                    s[("abds", w)] = abds
                return seg

            def mk_attn_b(w):
                # Z colsums, 1/Z, gate row, broadcast, scaled attn weights
                def seg():
                    wn = WINS[w]
                    L = wn * S
                    woS = wofs[w] * S
                    abds = s[("abds", w)]
                    psz = psr.tile([1, 512], F32, tag="psr")
                    for h in range(H):
                        nc.tensor.matmul(
                            psz[0:1, h * L:h * L + L],
                            lhsT=ones_col_bf[:L, 0:1],
                            rhs=abds[h][:L, :L],
                            start=True, stop=True,
                        )
                    rz = f32w.tile([1, 512], F32, tag="rz", bufs=2)
                    nc.vector.reciprocal(rz[0:1, :H * L], psz[0:1, :H * L])
                    wrow = f32w.tile([1, 512], F32, tag="wrow", bufs=2)
                    nc.vector.tensor_mul(
                        wrow[0:1, :H * L].rearrange("o (h m) -> o h m", m=L),
                        rz[0:1, :H * L].rearrange("o (h m) -> o h m", m=L),
                        s["growr"][0:1, :].rearrange(
                            "o (h m) -> o h m", m=MC)[:, :, woS:woS + L],
                    )
                    wbc = f32w.tile([P, 512], F32, tag="wbc", bufs=2)
                    for h in range(H):
                        nc.gpsimd.partition_broadcast(
                            wbc[:, h * L:h * L + L], wrow[0:1, h * L:h * L + L]
                        )
                    abscs = []
                    for h in range(H):
                        absc = act.tile([120, 128], BF16, tag="absc", bufs=6)
                        nc.vector.tensor_mul(
                            absc[:L, :L], abds[h][:L, :L], wbc[:L, h * L:h * L + L]
                        )
                        abscs.append(absc)
                    s[("abscs", w)] = abscs
                return seg

            def mk_attn_c(w, fts):
                # attnx for feature tiles in fts + evac into ysc
                def seg():
                    xw = s["xw"]
                    wn = WINS[w]
                    L = wn * S
                    woS = wofs[w] * S
                    abscs = s[("abscs", w)]
                    for ft in fts:
                        pyt = psy.tile([P, 512], F32, tag="psy")
                        for h in range(H):
                            nc.tensor.matmul(
                                pyt[:, h * L:h * L + L],
                                lhsT=xw[:L, w, ft * P:(ft + 1) * P],
                                rhs=abscs[h][:L, :L],
                                start=True, stop=True,
                            )
                        dst = ysc[:, :, woS:woS + L].rearrange(
                            "p (h f) m -> p h f m", f=FT)[:, :, ft, :]
                        src = pyt[:, :H * L].rearrange("p (h m) -> p h m", m=L)
                        if ft % 2 == 0:
                            nc.vector.tensor_copy(dst, src)
                        else:
                            nc.scalar.copy(dst, src)
                return seg

            for w in range(len(WINS)):
                segs.append(mk_attn_a(w))
                segs.append(mk_attn_b(w))
                segs.append(mk_attn_c(w, (0, 1)))
                segs.append(mk_attn_c(w, (2, 3)))

            # E: fused projection + residual (LN1 stats deferred one dp so
            # the PE never waits on the stt/sq of the dp it just produced)
            x1u = f32w.tile([P, DPT, MC], F32R, tag="x1u", bufs=1)
            sq1 = f32w.tile([P, DPT, MC], F32R, tag="sq1", bufs=1)
            s["x1u"], s["sq1"] = x1u, sq1

            def stats1(dp):
                nc.tensor.matmul(
                    s["pstat1"][0:1, :MC],
                    lhsT=ones_col_f32[:, 0:1],
                    rhs=x1u[:, dp, :],
                    start=(dp == 0), stop=(dp == DPT - 1),
                )
                nc.tensor.matmul(
                    s["psqs1"][0:1, :MC],
                    lhsT=ones_col_f32[:, 0:1],
                    rhs=sq1[:, dp, :],
                    start=(dp == 0), stop=(dp == DPT - 1),
                )

            def mk_proj(dp):
                def seg():
                    if dp == 0:
                        # allocated here (not at construction) so the psr
                        # rotation order matches emission order
                        s["pstat1"] = psr.tile([1, 512], F32, tag="psr", name="pstat1")
                        s["psqs1"] = psy.tile([1, 512], F32, tag="psy", name="psqs1")
                    xt = s["xt"]
                    ps = psb.tile([P, 512], F32, tag="psb")
                    for t in range(TT):
                        nc.tensor.matmul(
                            ps[:, :MC],
                            lhsT=wvp_sb[:, t, dp * P:(dp + 1) * P],
                            rhs=ysc[:, t, :],
                            start=(t == 0), stop=False,
                        )
                    nc.tensor.matmul(
                        ps[:, :MC],
                        lhsT=cg_sb[:, dp * P:(dp + 1) * P],
                        rhs=s["grep"][:, :],
                        start=False, stop=True,
                    )
                    nc.vector.scalar_tensor_tensor(
                        out=x1u[:, dp, :],
                        in0=ps[:, :MC],
                        scalar=bpc_sb[:, dp:dp + 1],
                        in1=xt[:, dp, :],
                        op0=ALU.add,
                        op1=ALU.add,
                    )
                    nc.scalar.activation(sq1[:, dp, :], x1u[:, dp, :], AF.Square)
                    if dp > 0:
                        stats1(dp - 1)
                return seg
            for dp in range(DPT):
                segs.append(mk_proj(dp))
            return segs

        # LN1 of chunk ch, emitted early in iteration ch+1 (3-stage pipeline)
        def mid_segments(ch):
            s = st[ch]
            x1n = act.tile([P, DPT, MC], BF16, tag="x1n", bufs=1)
            s["x1n"] = x1n

            def seg_e_tail():
                dp = DPT - 1
                nc.tensor.matmul(
                    s["pstat1"][0:1, :MC],
                    lhsT=ones_col_f32[:, 0:1],
                    rhs=s["x1u"][:, dp, :],
                    start=(dp == 0), stop=(dp == DPT - 1),
                )
                nc.tensor.matmul(
                    s["psqs1"][0:1, :MC],
                    lhsT=ones_col_f32[:, 0:1],
                    rhs=s["sq1"][:, dp, :],
                    start=(dp == 0), stop=(dp == DPT - 1),
                )

            def seg_ln1_chain():
                rs_bc, nm_bc = ln_chain(s["pstat1"], s["psqs1"], tag="1")
                s["rs1"], s["nm1"] = rs_bc, nm_bc

            def seg_ln1_apply():
                ln_apply(s["x1u"], s["rs1"], s["nm1"],
                         lambda dp: x1n[:, dp, :], 0)
            return [seg_e_tail, seg_ln1_chain, seg_ln1_apply]

        # shared LN helpers (T-layout; stats already in pstat rows {0, 32}).
        # rstd = rsqrt(var) via the quake bit trick + 2 Newton steps, all on
        # GPSIMD so the Act engine never leaves the exp table set (eps is
        # dropped: row variance here is O(1), so eps=1e-5 is far below the
        # bf16 noise floor).
        def ln_chain(pstat, psqs, tag):
            U32 = mybir.dt.uint32
            # negmean = -sum/D (sign is irrelevant for the square)
            mean = f32w.tile([1, 512], F32, tag="mean", bufs=1)
            nc.vector.tensor_scalar_mul(mean[0:1, :MC], pstat[0:1, :MC], -1.0 / D)
            msq = f32w.tile([1, 512], F32, tag="msq", bufs=1)
            nc.scalar.activation(msq[0:1, :MC], mean[0:1, :MC], AF.Square)
            var = f32w.tile([1, 512], F32, tag="var", bufs=1)
            nc.vector.scalar_tensor_tensor(
                out=var[0:1, :MC], in0=psqs[0:1, :MC], scalar=1.0 / D,
                in1=msq[0:1, :MC], op0=ALU.mult, op1=ALU.subtract,
            )
            std = f32w.tile([1, 512], F32, tag="std", bufs=1)
            nc.scalar.activation(std[0:1, :MC], var[0:1, :MC], AF.Sqrt,
                                 bias=eps_sb[0:1, 0:1])
            rs_row = f32w.tile([1, 512], F32, tag="rsr", bufs=1)
            nc.vector.reciprocal(rs_row[0:1, :MC], std[0:1, :MC])
            # nm = -mean*rstd = negmean*rstd
            nm_row = f32w.tile([1, 512], F32, tag="nmr", bufs=1)
            nc.gpsimd.tensor_mul(nm_row[0:1, :MC], mean[0:1, :MC], rs_row[0:1, :MC])
            rs_bc = f32w.tile([P, MC], F32, tag="rsb", bufs=1)
            nc.gpsimd.partition_broadcast(rs_bc[:, :], rs_row[0:1, :MC])
            nm_bc = f32w.tile([P, MC], F32, tag="nmb", bufs=1)
            nc.gpsimd.partition_broadcast(nm_bc[:, :], nm_row[0:1, :MC])
            return rs_bc, nm_bc

        def ln_apply(xu, rs_bc, nm_bc, dst, ln_row):
            for dp in range(DPT):
                tmp = f32w.tile([P, MC], F32R, tag="lntmp", bufs=2)
                nc.vector.tensor_mul(tmp[:, :], xu[:, dp, :], rs_bc[:, :])
                if apply_ln_affine:
                    t2 = f32w.tile([P, MC], F32R, tag="lnt2", bufs=2)
                    nc.gpsimd.tensor_add(t2[:, :], tmp[:, :], nm_bc[:, :])
                    nc.vector.tensor_scalar(
                        out=dst(dp), in0=t2[:, :],
                        scalar1=ln_sb[:, ln_row, dp:dp + 1],
                        scalar2=ln_sb[:, ln_row + 1, dp:dp + 1],
                        op0=ALU.mult, op1=ALU.add,
                    )
                else:
                    nc.gpsimd.tensor_add(dst(dp), tmp[:, :], nm_bc[:, :])

        # ---------------- back half: FFN, LN2, store -------------------------
        def back_segments(ch):
            s = st[ch]
            m0 = ch * MC
            segs = []
            pso = [psb.tile([P, 512], F32, tag="psb", name=f"pso{ch}_{i}")
                   for i in range(DPT)]
            s["pso"] = pso

            def ffn1_half(d1):
                x1n = s["x1n"]
                psf = psq.tile([P, 512], F32, tag="psq")
                for ft in range(FT):
                    nc.tensor.matmul(
                        psf[:, :MC],
                        lhsT=w1_sb[:, ft, d1 * P:(d1 + 1) * P],
                        rhs=x1n[:, ft, :],
                        start=(ft == 0),
                        stop=(ft == FT - 1),
                    )
                hrelu = act.tile([P, MC], BF16, tag="hrelu", bufs=6)
                if d1 % 2 == 0:
                    nc.vector.tensor_scalar(
                        out=hrelu[:, :], in0=psf[:, :MC],
                        scalar1=b1c_sb[:, d1:d1 + 1], scalar2=0.0,
                        op0=ALU.add, op1=ALU.max,
                    )
                else:
                    nc.scalar.activation(
                        hrelu[:, :], psf[:, :MC], AF.Relu,
                        bias=b1c_sb[:, d1:d1 + 1],
                    )
                return hrelu

            def ffn2_half(d1, hrelu):
                for dp in range(DPT):
                    nc.tensor.matmul(
                        pso[dp][:, :MC],
                        lhsT=w2_sb[:, d1, dp * P:(dp + 1) * P],
                        rhs=hrelu[:, :],
                        start=(d1 == 0),
                        stop=(d1 == D1T - 1),
                    )

            def mk_ffn_pair(d1):
                def seg():
                    ha = ffn1_half(d1)
                    hb = ffn1_half(d1 + 1)
                    ffn2_half(d1, ha)
                    ffn2_half(d1 + 1, hb)
                return seg
            for d1 in range(0, D1T, 2):
                segs.append(mk_ffn_pair(d1))

            # H1: residual + LN2 stats
            x2u = f32w.tile([P, DPT, MC], F32R, tag="x2u", bufs=1)
            sq2 = f32w.tile([P, DPT, MC], F32R, tag="sq2", bufs=1)
            x2n = f32w.tile([P, DPT, MC], F32, tag="x2n", bufs=1)

            def seg_h1a():
                x1n = s["x1n"]
                for dp in range(DPT):
                    nc.vector.scalar_tensor_tensor(
                        out=x2u[:, dp, :],
                        in0=pso[dp][:, :MC],
                        scalar=b2c_sb[:, dp:dp + 1],
                        in1=x1n[:, dp, :],
                        op0=ALU.add,
                        op1=ALU.add,
                    )
                    nc.scalar.activation(sq2[:, dp, :], x2u[:, dp, :], AF.Square)
            segs.append(seg_h1a)

            def seg_h1b():
                pstat2 = psr.tile([1, 512], F32, tag="psr", name="pstat2")
                psqs2 = psy.tile([1, 512], F32, tag="psy", name="psqs2")
                s["pstat2"], s["psqs2"] = pstat2, psqs2
                for dp in range(DPT):
                    nc.tensor.matmul(
                        pstat2[0:1, :MC],
                        lhsT=ones_col_f32[:, 0:1],
                        rhs=x2u[:, dp, :],
                        start=(dp == 0), stop=(dp == DPT - 1),
                    )
                    nc.tensor.matmul(
                        psqs2[0:1, :MC],
                        lhsT=ones_col_f32[:, 0:1],
                        rhs=sq2[:, dp, :],
                        start=(dp == 0), stop=(dp == DPT - 1),
                    )
            segs.append(seg_h1b)

            def seg_h2():
                rs_bc, nm_bc = ln_chain(s["pstat2"], s["psqs2"], tag="2")
                s["rs2"], s["nm2"] = rs_bc, nm_bc
            segs.append(seg_h2)

            def seg_h3():
                ln_apply(x2u, s["rs2"], s["nm2"],
                         lambda dp: x2n[:, dp, :], 2)
            segs.append(seg_h3)

            def seg_store():
                for dp in range(DPT):
                    nc.sync.dma_start(
                        out_flat[dp * P:(dp + 1) * P, m0:m0 + MC],
                        x2n[:, dp, :],
                    )
                st.pop(ch, None)
            segs.append(seg_store)
            return segs

        # ---------------- emission: 3-stage software pipeline ----------------
        # front seg indices: B=0..15, C=16, D=17..28 (4 per window), E=29..32
        # mid (ch-1): 0=E-tail stats, 1=LN1 chain, 2=LN1 apply
        # back (ch-1): G=0..15 (ffn d1), H1=16, H2=17, H3=18, I=19
        insert_after = {
            2: [("m", 0)],                              # LN1 dp3 stats
            3: [("m", 1)],                              # LN1 chain
            11: [("m", 2)],                             # LN1 apply -> x1n
            16: [("b", 0)], 17: [("b", 1)], 18: [("b", 2)], 19: [("b", 3)],
            20: [("b", 4)], 21: [("b", 5)], 22: [("b", 6)], 23: [("b", 7)],
            24: [("b", 8)],                             # H1a: x2u evac (DVE)
            25: [("b", 9)],                             # H1b: LN2 stats
            26: [("b", 10)],                            # H2: LN2 chain
            28: [("b", 11)],                            # H3: LN2 apply
            29: [("b", 12)],                            # store
        }

        load_xt(0)
        load_big_weights(0)      # m
        load_xw(0)
        load_big_weights(1)      # wvp
        if nch > 1:
            load_xt(1)
            load_xw(1)
        load_big_weights(2)      # w1, w2
        prev_back = None
        prev_ch = None
        for ch in range(nch):
            fsegs = front_segments(ch)
            mids = mid_segments(prev_ch) if prev_ch is not None else None
            if pipeline and prev_back is not None:
                done = set()
                for fi, fseg in enumerate(fsegs):
                    fseg()
                    for kind, bi in insert_after.get(fi, ()):
                        (mids if kind == "m" else prev_back)[bi]()
                        done.add((kind, bi))
                for bi in range(len(mids)):
                    if ("m", bi) not in done:
                        mids[bi]()
                for bi in range(len(prev_back)):
                    if ("b", bi) not in done:
                        prev_back[bi]()
            else:
                if mids is not None:
                    for mseg in mids:
                        mseg()
                if prev_back is not None:
                    for bseg in prev_back:
                        bseg()
                for fseg in fsegs:
                    fseg()
            if ch + 2 < nch:
                load_xt(ch + 2)
                load_xw(ch + 2)
            prev_back = back_segments(ch)
            prev_ch = ch
        for mseg in mid_segments(prev_ch):
            mseg()
        for bseg in prev_back:
            bseg()

        _stack.close()

    nc.compile()
    return nc


def _prep_inputs(inputs):
    """Host-side weight fusion; returns per-core in_maps."""
    bf = ml_dtypes.bfloat16
    x = np.ascontiguousarray(inputs["x"], dtype=np.float32)
    Wq = inputs["Wq"].astype(np.float32)
    Wk = inputs["Wk"].astype(np.float32)
    Wv = inputs["Wv"].astype(np.float32)
    Wp = inputs["Wp"].astype(np.float32).reshape(H, D, D)
    sc = 1.0 / math.sqrt(D)
    # M_h = Wq_h Wk_h^T / sqrt(D), stacked head-major on columns: [F, H*F]
    M = np.einsum("hfd,hgd->hfg", Wq, Wk) * sc
    m_p = np.ascontiguousarray(M.transpose(1, 0, 2).reshape(F, H * F)).astype(bf)
    # c_h = Wk_h bq_h / sqrt(D): per-key additive bias -> [F, H]
    cq_p = (np.einsum("hfd,hd->hf", Wk, inputs["bq"].astype(np.float32))
            * sc).T.astype(bf)
    cq_p = np.ascontiguousarray(cq_p)
    wvp_p = np.einsum("hfd,hde->hfe", Wv, Wp).reshape(H * F, D).astype(bf)
    cg_p = np.einsum("hd,hde->he", inputs["bv"].astype(np.float32), Wp).astype(bf)
    w1_p = inputs["W1"].astype(bf)
    w2_p = inputs["W2"].astype(bf)
    wg_p = (inputs["Wg"].astype(np.float32) / S).astype(bf)

    def col(v, nt):
        return np.ascontiguousarray(v.astype(np.float32).reshape(nt, 128).T)

    bpc_p = col(inputs["bp"], DPT)
    b1c_p = col(inputs["b1"], D1T)
    b2c_p = col(inputs["b2"], DPT)
    bg_p = inputs["bg"].astype(np.float32).reshape(1, H).astype(bf)
    ln_p = np.stack(
        [inputs["g1"], inputs["be1"], inputs["g2"], inputs["be2"]]
    ).astype(np.float32)
    apply_affine = not (
        np.all(ln_p[0] == 1) and np.all(ln_p[1] == 0)
        and np.all(ln_p[2] == 1) and np.all(ln_p[3] == 0)
    )
    shared = dict(
        m_p=m_p, cq_p=cq_p, wvp_p=wvp_p, cg_p=cg_p, w1_p=w1_p, w2_p=w2_p,
        wg_p=wg_p, bpc_p=bpc_p, b1c_p=b1c_p, b2c_p=b2c_p, bg_p=bg_p,
        mask_p=_make_mask(),
    )
    if apply_affine:
        shared["ln_p"] = ln_p
    x_bf = x.reshape(-1, F).astype(bf)
    in_maps = []
    for c in range(NCORES):
        m = dict(shared)
        m["x_bf"] = np.ascontiguousarray(x_bf[c * BC * S:(c + 1) * BC * S])
        in_maps.append(m)
    return in_maps, apply_affine


def _prep_inputs_small(inputs, nsamp):
    """Single map covering the first nsamp samples (for CoreSim tests)."""
    sub = dict(inputs)
    sub["x"] = np.asarray(inputs["x"])[:nsamp]
    maps, apply_affine = _prep_inputs(sub)
    m = maps[0]
    m["x_bf"] = m["x_bf"][: nsamp * S]
    return m, apply_affine


def _make_mask():
    m = np.zeros((120, 120), dtype=np.float32)
    for b in range(12):
        m[10 * b:10 * b + 10, 10 * b:10 * b + 10] = 1.0
    return m.astype(ml_dtypes.bfloat16)


_CACHED = {}


def _get_kernel(apply_affine):
    key = apply_affine
    if key not in _CACHED:
        _CACHED[key] = build_kernel(apply_affine)
    return _CACHED[key]


def kernel(**inputs):
    from concourse.bass_utils import run_bass_kernel_spmd

    in_maps, apply_affine = _prep_inputs(inputs)
    nc = _get_kernel(apply_affine)
    res = run_bass_kernel_spmd(nc, in_maps, list(range(NCORES)))
    outs = [
        np.asarray(r["out"]).reshape(D, BC * S).T.reshape(BC, S, F)
        for r in res.results
    ]
    return np.concatenate(outs, axis=0)


if __name__ == "__main__":
    nc = build_kernel(False)
    print("built ok")
